# revision 57
# baseline (speedup 1.0000x reference)
"""BertBlock kernel for 8 Trainium2 NeuronCores.

Sharding: pure data-parallel over (batch, half-sequence) tokens: core c
handles batch element c//2, query-token half c%2 (1024 tokens). Each core
recomputes K/V for the full 2048-token sequence of its batch element (the
duplicated K/V projection work is far cheaper than any 2-rank collective),
so no collectives are needed at all.

Device layout is feature-major ([feature, token]) end to end; the host
pre-transposes each core's x slice and rotates it so the core's own query
half always sits at columns 0:SQ (the program is SPMD-identical; key
order is irrelevant to the attention reduction). x is bf16 everywhere,
doubling as the attention residual. Softmax denominators come from
ones-columns in the attention-V stationary blocks, with even/odd heads
taking partition-aligned 128-col slices of a 161-col head-pair block so
normalization never crosses the partition-64 boundary. Per-head softmax
normalization is fully off the PE: the av accumulator is spilled by the
DVE, the reciprocal row computed on the DVE (Act exp(-ln d) for the last
head to shorten the tail), partition-broadcast on GpSimd, and the divide
deferred two heads so the PE never waits. LayerNorm stats matmuls are
interleaved into the producing loops; rstd comes from Act's
exp(-0.5*ln(var*H^2)) (ln/exp are table activations, far faster than the
DVE reciprocal) with the 1/H folded into pre-scaled affine weights; the
per-chunk normalize is split across GpSimd/DVE/Act. Large memsets live on
the Vector engine and weight-chunk DMA issues go first on GpSimd so the
PE starts within ~18us.
"""

import numpy as np
import ml_dtypes

P = 128
B = 4
S = 2048          # sequence length (keys)
SQ = 1024         # query tokens per core
H = 768
HC = H // P       # 6 feature chunks
NH = 12
DH = 64
FF = 3072
FC = FF // P      # 24
TS = S // P       # 16 key-token chunks
TQ = SQ // P      # 8 query-token chunks
N_CORES = 8
EPS = 1e-5
BF16 = ml_dtypes.bfloat16

_CACHE = {}


def _emit(nc, tc, t, mybir, make_identity):
    """Emit the per-core program. `t` maps tensor name -> DRAM AP."""
    from contextlib import ExitStack

    f32 = mybir.dt.float32
    f32r = mybir.dt.float32r
    bf16 = mybir.dt.bfloat16
    AF = mybir.ActivationFunctionType
    OP = mybir.AluOpType

    def mm(ps, lhsT, rhs, start, stop):
        nc.tensor.matmul(ps, lhsT=lhsT, rhs=rhs, start=start, stop=stop)

    with ExitStack() as ctx:
        aux = ctx.enter_context(tc.tile_pool(name="aux", bufs=1))

        # aux tiles are allocated up front but their (small) loads are
        # issued on the Sync queue after the xTq input DMAs: GpSimd must
        # stay free for the weight-chunk DMAs the PE waits on at startup.
        _aux_pending = []

        def aux_load(name, shape, dtype=f32):
            tl = aux.tile(shape, dtype, tag=name)
            _aux_pending.append((tl, t[name]))
            return tl

        def flush_aux():
            for tl, src in _aux_pending:
                nc.sync.dma_start(tl[:], src)
            _aux_pending.clear()

        bq_s = aux_load("bq2", [P, HC])
        bk_s = aux_load("bk2", [P, HC])
        bo_s = aux_load("bo2", [P, HC])
        b2_s = aux_load("b22", [P, HC])
        l1w_s = aux_load("l1w", [P, HC])
        l1b_s = aux_load("l1b", [P, HC])
        l2w_s = aux_load("l2w", [P, HC])
        l2b_s = aux_load("l2b", [P, HC])
        b1_s = aux_load("b12", [P, FC])
        bvb_s = aux.tile([P, H], f32)
        _aux_pending.append((bvb_s, t["bv"].partition_broadcast(P)))
        ones_f = aux.tile([P, 1], f32)
        nc.vector.memset(ones_f[:], 1.0)
        ones_s = aux.tile([P, 1], f32r)
        nc.vector.tensor_copy(ones_s[:], ones_f[:])
        zero_s = aux.tile([P, 1], f32)
        nc.vector.memset(zero_s[:], 0.0)
        epsh_s = aux.tile([1, 1], f32)
        nc.vector.memset(epsh_s[:], EPS * H * H)
        # LN affine weights pre-scaled by H (ln_rows computes rstd/H);
        # filled right before the O-projection so the startup vector queue
        # stays clear of aux-DMA dependencies.
        l1wH_s = aux.tile([P, HC], f32)
        l2wH_s = aux.tile([P, HC], f32)
        # identity for the output transposes: bf16, and its gpsimd emission
        # is deferred until the MLP phase (see below) so it never delays the
        # startup weight DMAs.
        ident_s = aux.tile([P, P], bf16)

        # x1 (LN1 output, bf16) outlives the attention/O-proj scopes below.
        # bf16 is used both as the MLP input and the residual-2 operand; the
        # ~0.4% rounding is far inside the tolerance budget.
        keep = ctx.enter_context(tc.tile_pool(name="keep", bufs=1))
        x1b_s = keep.tile([P, HC, SQ], bf16)
        # weight-stream pools live low in SBUF so their DMAs never alias the
        # attention-phase pools and can prefetch during earlier phases
        wop = ctx.enter_context(tc.tile_pool(name="wo_st", bufs=3))
        w1p = ctx.enter_context(tc.tile_pool(name="w1_st", bufs=3))

        def ln_rows(pool, sum_ps, sq_ps):
            """Turn the accumulated sum/sq-sum psum rows into partition-
            broadcast mean/rstd' tiles. The critical chain avoids both the
            mean dependency (var·H² = sq·H − sum²) and the slow DVE
            reciprocal (rstd' = exp(−½·ln(varH2+εH²)) on Act; ln and exp
            share an activation table). rstd' = rstd/H — the missing ×H is
            folded into the affine weight (see *_wH tiles)."""
            # mean first: the mb broadcast (GpSimd) overlaps the Act/DVE
            # variance chain, so the first t1 never waits on it
            m2r = pool.tile([1, SQ], f32, tag="lnsc", bufs=2)
            nc.scalar.activation(m2r[:], sum_ps[:], AF.Square)
            mean = pool.tile([1, SQ], f32r, tag="lnmean", bufs=1)
            nc.vector.tensor_scalar_mul(mean[:], sum_ps[:], 1.0 / H)
            mb = pool.tile([P, SQ], f32r, tag="lnmb", bufs=1)
            nc.gpsimd.partition_broadcast(mb[:], mean[:], channels=P)
            varh = pool.tile([1, SQ], f32, tag="lnsc", bufs=2)
            nc.vector.scalar_tensor_tensor(
                out=varh[:], in0=sq_ps[:], scalar=float(H), in1=m2r[:],
                op0=OP.mult, op1=OP.subtract,
            )
            lnv = pool.tile([1, SQ], f32, tag="lnsc", bufs=2)
            nc.scalar.activation(lnv[:], varh[:], AF.Ln, bias=epsh_s[:])
            rstd = pool.tile([1, SQ], f32r, tag="lnrstd", bufs=1)
            with nc.allow_low_precision(reason="act-table rstd is benign"):
                nc.scalar.activation(rstd[:], lnv[:], AF.Exp, scale=-0.5)
            rb = pool.tile([P, SQ], f32r, tag="lnrb", bufs=1)
            nc.gpsimd.partition_broadcast(rb[:], rstd[:], channels=P)
            return mb, rb

        def ln_chunks(pool, src, mb, rb, emit_chunk):
            """Per-chunk normalization: subtract on GpSimd, multiply on DVE
            (pipelined); `emit_chunk(j, t2)` emits the affine + stores."""
            for j in range(HC):
                t1 = pool.tile([P, SQ], f32, tag="lnt1", bufs=2)
                nc.gpsimd.tensor_sub(t1[:], src[:, j, :], mb[:])
                t2 = pool.tile([P, SQ], f32, tag="lnt2", bufs=2)
                nc.vector.tensor_tensor(t2[:], t1[:], rb[:], OP.mult)
                emit_chunk(j, t2)

        with tc.tile_pool(name="resid", bufs=1) as resid:
            # xT lives here (not in the QKV scope): columns 0:SQ are this
            # core's query tokens and double as the attention residual.
            xT_s = resid.tile([P, HC, S], bf16)
            xt_src = t["xT"].rearrange("(c p) s -> p c s", p=P)
            for j in range(HC):
                eng = nc.sync if j % 2 == 0 else nc.scalar
                eng.dma_start(xT_s[:, j, :], xt_src[:, j, :])
            flush_aux()
            with tc.tile_pool(name="attn_out", bufs=1) as aop:
                attnT_s = aop.tile([P, HC, SQ], bf16)

                with tc.tile_pool(name="qkv_keep", bufs=1) as p2:
                    # qTz[p, h, q]: head h's 64 q-rows live at partitions
                    # (h%2)*64..+64 of plane h; the other 64 partitions stay
                    # zero so scores can contract over all 128 partitions
                    # (full PE-array activity keeps the HAM clock warm).
                    qTz_s = p2.tile([P, NH, SQ], bf16)
                    kT_s = p2.tile([P, HC, S], bf16)
                    # v_s head-pair blocks of 161 columns:
                    #   [V_even(0:64) | ones_e(64) | ones_o(65) | 0(66:97) |
                    #    V_odd(97:161)]
                    # Even head 2j takes the 128-col stationary slice at
                    # 161j+0 (V rows -> psum partitions 0:64, denominator at
                    # 64); odd head 2j+1 takes the slice at 161j+33 (V rows
                    # -> partitions 64:128, denominator at partition 32 —
                    # engine partition accesses must start at multiples of
                    # 32). This keeps every head's attn rows partition-
                    # aligned with its attnT destination, so normalization
                    # never needs a partition-shifting DMA.
                    VB = 161
                    v_s = p2.tile([P, TS, VB * HC], bf16)
                    v_view = v_s[:].rearrange("p t (j c) -> p t j c", j=HC)

                    # ---------------- QKV projections ----------------
                    with tc.tile_pool(
                        name="wstream", bufs=3
                    ) as ws, tc.tile_pool(
                        name="qkv_ps", bufs=3, space="PSUM"
                    ) as pp:
                        # zero-fill memsets on the (otherwise idle) Vector
                        # engine, split per plane-half so the first Q bias
                        # write only waits for its own planes.
                        for j in range(HC):
                            nc.vector.memset(qTz_s[DH:P, 2 * j, :], 0.0)
                            nc.vector.memset(qTz_s[0:DH, 2 * j + 1, :], 0.0)
                        nc.vector.memset(v_view[:, :, :, DH : DH + 2], 1.0)
                        nc.vector.memset(v_view[:, :, :, DH + 2 : 97], 0.0)

                        # Q (our 1024 query tokens = xT columns 0:SQ)
                        for j in range(HC):
                            w_t = ws.tile([P, HC, P], bf16, tag="w")
                            nc.gpsimd.dma_start(
                                w_t[:],
                                t["Wq"][:, j * P : (j + 1) * P].rearrange(
                                    "(c p) m -> p c m", p=P
                                ),
                            )
                            ps = pp.tile([P, SQ], f32, tag="qkps")
                            for kc in range(HC):
                                for n in range(2):
                                    mm(
                                        ps[:, n * 512 : (n + 1) * 512],
                                        w_t[:, kc, :],
                                        xT_s[:, kc, n * 512 : (n + 1) * 512],
                                        kc == 0,
                                        kc == HC - 1,
                                    )
                            nc.scalar.activation(
                                qTz_s[0:DH, 2 * j, :], ps[0:DH, :],
                                AF.Identity, bias=bq_s[0:DH, j : j + 1],
                            )
                            nc.scalar.activation(
                                qTz_s[DH:P, 2 * j + 1, :], ps[DH:P, :],
                                AF.Identity, bias=bq_s[DH:P, j : j + 1],
                            )

                        # K (all 2048 tokens, bf16)
                        for j in range(HC):
                            wk_t = ws.tile([P, HC, P], bf16, tag="w")
                            nc.gpsimd.dma_start(
                                wk_t[:],
                                t["Wk"][:, j * P : (j + 1) * P].rearrange(
                                    "(c p) m -> p c m", p=P
                                ),
                            )
                            # kc outer over both sequence halves: one
                            # stationary load serves 4 matmuls
                            psk0 = pp.tile([P, SQ], f32, tag="qkps")
                            psk1 = pp.tile([P, SQ], f32, tag="qkps")
                            psk = [psk0, psk1]
                            for kc in range(HC):
                                for hf in range(2):
                                    for n in range(2):
                                        mm(
                                            psk[hf][:, n * 512 : (n + 1) * 512],
                                            wk_t[:, kc, :],
                                            xT_s[
                                                :, kc,
                                                hf * SQ + n * 512 :
                                                hf * SQ + (n + 1) * 512,
                                            ],
                                            kc == 0,
                                            kc == HC - 1,
                                        )
                            for hf in range(2):
                                nc.scalar.activation(
                                    kT_s[:, j, hf * SQ : (hf + 1) * SQ],
                                    psk[hf][:],
                                    AF.Identity,
                                    bias=bk_s[:, j : j + 1],
                                )

                        # V (token-major with per-head ones column)
                        wv_t = ws.tile([P, HC, H], bf16, tag="wv", bufs=1)
                        nc.gpsimd.dma_start(
                            wv_t[:], t["Wv"].rearrange("(c p) m -> p c m", p=P)
                        )
                        for tt in range(TS):
                            ps = pp.tile([P, SQ], f32, tag="qkps")
                            for kc in range(HC):
                                mm(
                                    ps[:, 0:512],
                                    xT_s[:, kc, tt * P : (tt + 1) * P],
                                    wv_t[:, kc, 0:512],
                                    kc == 0,
                                    kc == HC - 1,
                                )
                                mm(
                                    ps[:, 512:H],
                                    xT_s[:, kc, tt * P : (tt + 1) * P],
                                    wv_t[:, kc, 512:H],
                                    kc == 0,
                                    kc == HC - 1,
                                )
                            ps_v = ps[:, 0:H].rearrange(
                                "p (j two d) -> p j two d", j=HC, two=2
                            )
                            bv_v = bvb_s[:].rearrange(
                                "p (j two d) -> p j two d", j=HC, two=2
                            )
                            nc.vector.scalar_tensor_tensor(
                                out=v_view[:, tt, :, 0:DH],
                                in0=ps_v[:, :, 0, :],
                                scalar=1.0,
                                in1=bv_v[:, :, 0, :],
                                op0=OP.mult,
                                op1=OP.add,
                            )
                            nc.vector.scalar_tensor_tensor(
                                out=v_view[:, tt, :, 97:161],
                                in0=ps_v[:, :, 1, :],
                                scalar=1.0,
                                in1=bv_v[:, :, 1, :],
                                op0=OP.mult,
                                op1=OP.add,
                            )

                    # ---------------- attention ----------------
                    with tc.tile_pool(name="attn_sb", bufs=1) as ab, tc.tile_pool(
                        name="probs", bufs=4
                    ) as prp, tc.tile_pool(
                        name="sc_ps", bufs=2, space="PSUM"
                    ) as pps, tc.tile_pool(
                        name="av_ps", bufs=2, space="PSUM"
                    ) as ppa:
                        avs = {}
                        spills = {}
                        bcs = {}

                        def spill_head(h):
                            # Copy the raw accumulator (attn rows + sums row,
                            # already partition-aligned with attnT) to SBUF on
                            # the DVE, compute the reciprocal row, broadcast
                            # it on GpSimd. For the last two heads the
                            # reciprocal runs on the (by-then idle) Act engine
                            # as exp(−ln d) — ~2.4µs instead of the 7.8µs DVE
                            # reciprocal, shortening the attention tail.
                            av = avs.pop(h)
                            avs_sb = ab.tile([P, SQ], f32, tag="avsb", bufs=3)
                            if h % 2 == 0:
                                dlo, dhi = 0, DH
                                drow = DH
                            else:
                                dlo, dhi = DH, P
                                drow = 32
                            nc.vector.tensor_copy(
                                avs_sb[dlo:dhi, :], av[dlo:dhi, :]
                            )
                            spills[h] = avs_sb
                            # the denominator row is read straight from PSUM
                            rec = ab.tile([1, SQ], f32r, tag="rec", bufs=3)
                            with nc.allow_low_precision(
                                reason="softmax recip rounding is benign"
                            ):
                                if h >= NH - 1:
                                    lnd = ab.tile(
                                        [1, SQ], f32, tag="lnd", bufs=2
                                    )
                                    nc.scalar.activation(
                                        lnd[:], av[drow : drow + 1, :],
                                        AF.Ln,
                                    )
                                    nc.scalar.activation(
                                        rec[:], lnd[:], AF.Exp, scale=-1.0
                                    )
                                else:
                                    nc.vector.reciprocal(
                                        rec[:], av[drow : drow + 1, :]
                                    )
                            bc = ab.tile([P, SQ], f32r, tag="bcs", bufs=3)
                            nc.gpsimd.partition_broadcast(
                                bc[:], rec[:], channels=P
                            )
                            bcs[h] = bc

                        def normalize_head(h):
                            """Divide head h's attention rows by the softmax
                            sums and place them into attnT. Emitted two heads
                            behind the matmul stream, entirely off the PE,
                            partition-aligned for both parities."""
                            hc = h // 2
                            avs_sb = spills.pop(h)
                            bc = bcs.pop(h)
                            if h % 2 == 0:
                                nc.vector.tensor_tensor(
                                    attnT_s[0:DH, hc, :], avs_sb[0:DH, :],
                                    bc[0:DH, :], OP.mult,
                                )
                            else:
                                nc.vector.tensor_tensor(
                                    attnT_s[DH:P, hc, :], avs_sb[DH:P, :],
                                    bc[DH:P, :], OP.mult,
                                )

                        def emit_av(h, av, kt, pr):
                            base = VB * (h // 2) + (0 if h % 2 == 0 else 33)
                            for n in range(2):
                                mm(
                                    av[:, n * 512 : (n + 1) * 512],
                                    v_s[:, kt, base : base + P],
                                    pr[:, n * 512 : (n + 1) * 512],
                                    kt == 0,
                                    kt == TS - 1,
                                )

                        for h in range(NH):
                            hc = h // 2
                            av = ppa.tile([P, SQ], f32, tag="av")
                            avs[h] = av
                            pending = []
                            for kt in range(TS):
                                sc = pps.tile([P, SQ], f32, tag="sc")
                                lhsT_k = kT_s[
                                    :, hc, kt * P : (kt + 1) * P
                                ]
                                for n in range(2):
                                    mm(
                                        sc[:, n * 512 : (n + 1) * 512],
                                        lhsT_k,
                                        qTz_s[
                                            :, h, n * 512 : (n + 1) * 512
                                        ],
                                        True,
                                        True,
                                    )
                                pr = prp.tile([P, SQ], bf16, tag="pr")
                                nc.scalar.activation(
                                    pr[:], sc[:], AF.Exp, bias=zero_s[:],
                                    scale=0.125,
                                )
                                pending.append((kt, pr))
                                if len(pending) > 2:
                                    emit_av(h, av, *pending.pop(0))
                            for p_ in pending:
                                emit_av(h, av, *p_)
                            spill_head(h)
                            if h >= 2:
                                normalize_head(h - 2)
                        normalize_head(NH - 2)
                        normalize_head(NH - 1)

                # ------------- O-projection + residual + LN1 -------------
                with tc.tile_pool(name="oproj", bufs=1) as op_, tc.tile_pool(
                    name="o_ps", bufs=2, space="PSUM"
                ) as ppo, tc.tile_pool(
                    name="st_ps", bufs=1, space="PSUM"
                ) as ppst:
                    nc.vector.tensor_scalar_mul(
                        l1wH_s[:], l1w_s[:], float(H)
                    )
                    nc.vector.tensor_scalar_mul(
                        l2wH_s[:], l2w_s[:], float(H)
                    )
                    r1_s = op_.tile([P, HC, SQ], f32r)
                    sum_ps = ppst.tile([1, SQ], f32, tag="lnsum", bufs=1)
                    sq_ps = ppst.tile([1, SQ], f32, tag="lnsq", bufs=1)
                    for j in range(HC):
                        wo_t = wop.tile([P, HC, P], bf16, tag="wo")
                        nc.gpsimd.dma_start(
                            wo_t[:],
                            t["Wo"][:, j * P : (j + 1) * P].rearrange(
                                "(c p) m -> p c m", p=P
                            ),
                        )
                        ps = ppo.tile([P, SQ], f32, tag="ops")
                        for kc in range(HC):
                            for n in range(2):
                                mm(
                                    ps[:, n * 512 : (n + 1) * 512],
                                    wo_t[:, kc, :],
                                    attnT_s[
                                        :, kc, n * 512 : (n + 1) * 512
                                    ],
                                    kc == 0,
                                    kc == HC - 1,
                                )
                        nc.vector.scalar_tensor_tensor(
                            out=r1_s[:, j, :],
                            in0=ps[:],
                            scalar=bo_s[:, j : j + 1],
                            in1=xT_s[:, j, 0:SQ],
                            op0=OP.add,
                            op1=OP.add,
                        )
                        # LN1 stats accumulate as the chunks appear
                        sq_t = op_.tile([P, SQ], f32r, tag="lnsqt", bufs=2)
                        nc.vector.tensor_tensor(
                            sq_t[:], r1_s[:, j, :], r1_s[:, j, :], OP.mult
                        )
                        for n in range(2):
                            mm(
                                sum_ps[:, n * 512 : (n + 1) * 512],
                                ones_s[:],
                                r1_s[:, j, n * 512 : (n + 1) * 512],
                                j == 0,
                                j == HC - 1,
                            )
                            mm(
                                sq_ps[:, n * 512 : (n + 1) * 512],
                                ones_s[:],
                                sq_t[:, n * 512 : (n + 1) * 512],
                                j == 0,
                                j == HC - 1,
                            )
                    def ln1_chunk(j, t2):
                        nc.scalar.activation(
                            x1b_s[:, j, :], t2[:], AF.Identity,
                            scale=l1wH_s[:, j : j + 1],
                            bias=l1b_s[:, j : j + 1],
                        )

                    # prefetch the first W1 chunks now: the GpSimd queue is
                    # about to fill with LN1 work, and MLP1's first matmuls
                    # should only wait on x1b chunks, not weights
                    w1_pre = []
                    for m in range(3):
                        w1_t = w1p.tile([P, HC, P], bf16, tag="w1")
                        nc.gpsimd.dma_start(
                            w1_t[:],
                            t["W1"][:, m * P : (m + 1) * P].rearrange(
                                "(c p) n -> p c n", p=P
                            ),
                        )
                        w1_pre.append(w1_t)

                    mb1, rb1 = ln_rows(op_, sum_ps, sq_ps)
                    ln_chunks(op_, r1_s, mb1, rb1, ln1_chunk)

        # ---------------- MLP + LN2 + output ----------------
        with tc.tile_pool(name="mlp", bufs=1) as mp:
            hT_s = mp.tile([P, FC, SQ], bf16)
            r2_s = mp.tile([P, HC, SQ], f32r)
            w2_s = mp.tile([P, FC, H], bf16)
            # W2 prefetch: issue early on the idle Sync queue, chunked so
            # the first MLP2 matmul doesn't wait on the whole 4.7MB.
            w2_src = t["W2"].rearrange("(c p) m -> p c m", p=P)
            for ci in range(4):
                nc.sync.dma_start(
                    w2_s[:, ci * 6 : (ci + 1) * 6, :],
                    w2_src[:, ci * 6 : (ci + 1) * 6, :],
                )
            with tc.tile_pool(
                name="m_ps", bufs=2, space="PSUM"
            ) as ppm, tc.tile_pool(
                name="st2_ps", bufs=1, space="PSUM"
            ) as ppst2:
                for m in range(FC):
                    if m < len(w1_pre):
                        w1_t = w1_pre[m]
                    else:
                        w1_t = w1p.tile([P, HC, P], bf16, tag="w1")
                        nc.gpsimd.dma_start(
                            w1_t[:],
                            t["W1"][:, m * P : (m + 1) * P].rearrange(
                                "(c p) n -> p c n", p=P
                            ),
                        )
                    ps = ppm.tile([P, SQ], f32, tag="mps")
                    for kc in range(HC):
                        for n in range(2):
                            mm(
                                ps[:, n * 512 : (n + 1) * 512],
                                w1_t[:, kc, :],
                                x1b_s[:, kc, n * 512 : (n + 1) * 512],
                                kc == 0,
                                kc == HC - 1,
                            )
                    nc.scalar.activation(
                        hT_s[:, m, :], ps[:], AF.Gelu, bias=b1_s[:, m : m + 1]
                    )

                # identity for the output transposes (gpsimd is idle here)
                make_identity(nc, ident_s[:])

                sum2_ps = ppst2.tile([1, SQ], f32, tag="ln2sum", bufs=1)
                sq2_ps = ppst2.tile([1, SQ], f32, tag="ln2sq", bufs=1)
                for j in range(HC):
                    ps = ppm.tile([P, SQ], f32, tag="mps")
                    for kc in range(FC):
                        for n in range(2):
                            mm(
                                ps[:, n * 512 : (n + 1) * 512],
                                w2_s[:, kc, j * P : (j + 1) * P],
                                hT_s[:, kc, n * 512 : (n + 1) * 512],
                                kc == 0,
                                kc == FC - 1,
                            )
                    nc.vector.scalar_tensor_tensor(
                        out=r2_s[:, j, :],
                        in0=ps[:],
                        scalar=b2_s[:, j : j + 1],
                        in1=x1b_s[:, j, :],
                        op0=OP.add,
                        op1=OP.add,
                    )
                    sq_t = mp.tile([P, SQ], f32r, tag="ln2sqt", bufs=1)
                    nc.vector.tensor_tensor(
                        sq_t[:], r2_s[:, j, :], r2_s[:, j, :], OP.mult
                    )
                    for n in range(2):
                        mm(
                            sum2_ps[:, n * 512 : (n + 1) * 512],
                            ones_s[:],
                            r2_s[:, j, n * 512 : (n + 1) * 512],
                            j == 0,
                            j == HC - 1,
                        )
                        mm(
                            sq2_ps[:, n * 512 : (n + 1) * 512],
                            ones_s[:],
                            sq_t[:, n * 512 : (n + 1) * 512],
                            j == 0,
                            j == HC - 1,
                        )
                mb2, rb2 = ln_rows(mp, sum2_ps, sq2_ps)

            # LN2 chunks + transpose back to token-major + store, pipelined
            # per feature chunk (the MLP psum pools are closed here, freeing
            # banks for the transpose pool)
            with tc.tile_pool(name="outp", bufs=1) as outp, tc.tile_pool(
                name="tr_ps", bufs=4, space="PSUM"
            ) as ppt:

                def ln2_chunk(j, t2):
                    r2n = outp.tile([P, SQ], bf16, tag="r2n", bufs=2)
                    nc.vector.tensor_scalar(
                        r2n[:], t2[:], l2wH_s[:, j : j + 1],
                        l2b_s[:, j : j + 1], OP.mult, OP.add,
                    )
                    stage = outp.tile([P, TQ, P], f32, tag="out", bufs=2)
                    for tp in range(TQ // 2):
                        tps = ppt.tile([P, 2, P], bf16, tag="tr")
                        for k in range(2):
                            nc.tensor.transpose(
                                tps[:, k, :],
                                r2n[:, (2 * tp + k) * P : (2 * tp + k + 1) * P],
                                ident_s[:],
                            )
                        nc.scalar.activation(
                            stage[:, 2 * tp : 2 * tp + 2, :], tps[:],
                            AF.Identity, bias=zero_s[:],
                        )
                    eng = nc.sync if j % 2 == 0 else nc.scalar
                    eng.dma_start(
                        t["y"][:, j * P : (j + 1) * P].rearrange(
                            "(t p) f -> p t f", p=P
                        ),
                        stage[:],
                    )

                ln_chunks(mp, r2_s, mb2, rb2, ln2_chunk)


def _build():
    import concourse.bacc as bacc
    import concourse.tile as tile
    import concourse.mybir as mybir
    from concourse.masks import make_identity

    f32 = mybir.dt.float32
    f32r = mybir.dt.float32r
    bf16 = mybir.dt.bfloat16

    nc = bacc.Bacc(
        "TRN2", target_bir_lowering=False, debug=False, num_devices=N_CORES
    )
    specs = [
        ("xT", [H, S], bf16, "ExternalInput"),
        ("Wq", [H, H], bf16, "ExternalInput"),
        ("Wk", [H, H], bf16, "ExternalInput"),
        ("Wv", [H, H], bf16, "ExternalInput"),
        ("Wo", [H, H], bf16, "ExternalInput"),
        ("W1", [H, FF], bf16, "ExternalInput"),
        ("W2", [FF, H], bf16, "ExternalInput"),
        ("bq2", [P, HC], f32, "ExternalInput"),
        ("bk2", [P, HC], f32, "ExternalInput"),
        ("bv", [H], f32, "ExternalInput"),
        ("bo2", [P, HC], f32, "ExternalInput"),
        ("b12", [P, FC], f32, "ExternalInput"),
        ("b22", [P, HC], f32, "ExternalInput"),
        ("l1w", [P, HC], f32, "ExternalInput"),
        ("l1b", [P, HC], f32, "ExternalInput"),
        ("l2w", [P, HC], f32, "ExternalInput"),
        ("l2b", [P, HC], f32, "ExternalInput"),
        ("y", [SQ, H], f32, "ExternalOutput"),
    ]
    t = {
        name: nc.dram_tensor(name, shape, dt, kind=kind).ap()
        for name, shape, dt, kind in specs
    }
    with tile.TileContext(nc) as tc:
        _emit(nc, tc, t, mybir, make_identity)
    nc.compile()
    return nc


def _chunk_major(v):
    """[C*P] -> [P, C] with entry [p, c] = v[c*P + p]."""
    return np.ascontiguousarray(v.reshape(-1, P).T)


def prepare_in_maps(inputs):
    inp = {k: np.asarray(v) for k, v in inputs.items()}
    x = inp["x"].astype(np.float32)

    shared = {
        "Wq": inp["Wq"].astype(BF16),
        "Wk": inp["Wk"].astype(BF16),
        "Wv": inp["Wv"].astype(BF16),
        "Wo": inp["Wo"].astype(BF16),
        "W1": inp["W1"].astype(BF16),
        "W2": inp["W2"].astype(BF16),
        "bq2": _chunk_major(inp["bq"].astype(np.float32)),
        "bk2": _chunk_major(inp["bk"].astype(np.float32)),
        "bv": inp["bv"].astype(np.float32),
        "bo2": _chunk_major(inp["bo"].astype(np.float32)),
        "b12": _chunk_major(inp["b1"].astype(np.float32)),
        "b22": _chunk_major(inp["b2"].astype(np.float32)),
        "l1w": _chunk_major(inp["ln1_w"].astype(np.float32)),
        "l1b": _chunk_major(inp["ln1_b"].astype(np.float32)),
        "l2w": _chunk_major(inp["ln2_w"].astype(np.float32)),
        "l2b": _chunk_major(inp["ln2_b"].astype(np.float32)),
    }
    in_maps = []
    for c in range(N_CORES):
        b, hf = c // 2, c % 2
        xT = x[b].T
        if hf:
            # rotate so this core's query half sits at columns 0:SQ (the
            # program is SPMD-identical; key order is irrelevant since
            # attention reduces over all keys)
            xT = np.concatenate([xT[:, SQ:], xT[:, :SQ]], axis=1)
        m = dict(shared)
        m["xT"] = np.ascontiguousarray(xT).astype(BF16)
        in_maps.append(m)
    return in_maps


def get_program():
    if "nc" not in _CACHE:
        _CACHE["nc"] = _build()
    return _CACHE["nc"]


def kernel(**inputs):
    from concourse.bass_utils import run_bass_kernel_spmd

    nc = get_program()
    in_maps = prepare_in_maps(inputs)
    res = run_bass_kernel_spmd(nc, in_maps, core_ids=list(range(N_CORES)))
    out = np.empty((B, S, H), np.float32)
    for c in range(N_CORES):
        b, hf = c // 2, c % 2
        out[b, hf * SQ : (hf + 1) * SQ] = res.results[c]["y"]
    return out


# revision 58
# speedup vs baseline: 1.0007x; 1.0007x over previous
"""BertBlock kernel for 8 Trainium2 NeuronCores.

Sharding: pure data-parallel over (batch, half-sequence) tokens: core c
handles batch element c//2, query-token half c%2 (1024 tokens). Each core
recomputes K/V for the full 2048-token sequence of its batch element (the
duplicated K/V projection work is far cheaper than any 2-rank collective),
so no collectives are needed at all.

Device layout is feature-major ([feature, token]) end to end; the host
pre-transposes each core's x slice and rotates it so the core's own query
half always sits at columns 0:SQ (the program is SPMD-identical; key
order is irrelevant to the attention reduction). x is bf16 everywhere,
doubling as the attention residual. Softmax denominators come from
ones-columns in the attention-V stationary blocks, with even/odd heads
taking partition-aligned 128-col slices of a 161-col head-pair block so
normalization never crosses the partition-64 boundary. Per-head softmax
normalization is fully off the PE: the av accumulator is spilled by the
DVE, the reciprocal row computed on the DVE (Act exp(-ln d) for the last
head to shorten the tail), partition-broadcast on GpSimd, and the divide
deferred two heads so the PE never waits. LayerNorm stats matmuls are
interleaved into the producing loops; rstd comes from Act's
exp(-0.5*ln(var*H^2)) (ln/exp are table activations, far faster than the
DVE reciprocal) with the 1/H folded into pre-scaled affine weights; the
per-chunk normalize is split across GpSimd/DVE/Act. Large memsets live on
the Vector engine and weight-chunk DMA issues go first on GpSimd so the
PE starts within ~18us.
"""

import numpy as np
import ml_dtypes

P = 128
B = 4
S = 2048          # sequence length (keys)
SQ = 1024         # query tokens per core
H = 768
HC = H // P       # 6 feature chunks
NH = 12
DH = 64
FF = 3072
FC = FF // P      # 24
TS = S // P       # 16 key-token chunks
TQ = SQ // P      # 8 query-token chunks
N_CORES = 8
EPS = 1e-5
BF16 = ml_dtypes.bfloat16

_CACHE = {}


def _emit(nc, tc, t, mybir, make_identity):
    """Emit the per-core program. `t` maps tensor name -> DRAM AP."""
    from contextlib import ExitStack

    f32 = mybir.dt.float32
    f32r = mybir.dt.float32r
    bf16 = mybir.dt.bfloat16
    AF = mybir.ActivationFunctionType
    OP = mybir.AluOpType

    def mm(ps, lhsT, rhs, start, stop):
        nc.tensor.matmul(ps, lhsT=lhsT, rhs=rhs, start=start, stop=stop)

    with ExitStack() as ctx:
        aux = ctx.enter_context(tc.tile_pool(name="aux", bufs=1))

        # aux tiles are allocated up front but their (small) loads are
        # issued on the Sync queue after the xTq input DMAs: GpSimd must
        # stay free for the weight-chunk DMAs the PE waits on at startup.
        _aux_pending = []

        def aux_load(name, shape, dtype=f32):
            tl = aux.tile(shape, dtype, tag=name)
            _aux_pending.append((tl, t[name]))
            return tl

        def flush_aux():
            for tl, src in _aux_pending:
                nc.sync.dma_start(tl[:], src)
            _aux_pending.clear()

        bq_s = aux_load("bq2", [P, HC])
        bk_s = aux_load("bk2", [P, HC])
        bo_s = aux_load("bo2", [P, HC])
        b2_s = aux_load("b22", [P, HC])
        l1w_s = aux_load("l1w", [P, HC])
        l1b_s = aux_load("l1b", [P, HC])
        l2w_s = aux_load("l2w", [P, HC])
        l2b_s = aux_load("l2b", [P, HC])
        b1_s = aux_load("b12", [P, FC])
        bvb_s = aux.tile([P, H], f32)
        _aux_pending.append((bvb_s, t["bv"].partition_broadcast(P)))
        ones_s = aux.tile([P, 1], bf16)
        nc.vector.memset(ones_s[:], 1.0)
        zero_s = aux.tile([P, 1], f32)
        nc.vector.memset(zero_s[:], 0.0)
        epsh_s = aux.tile([1, 1], f32)
        nc.vector.memset(epsh_s[:], EPS * H * H)
        # LN affine weights pre-scaled by H (ln_rows computes rstd/H);
        # filled right before the O-projection so the startup vector queue
        # stays clear of aux-DMA dependencies.
        l1wH_s = aux.tile([P, HC], f32)
        l2wH_s = aux.tile([P, HC], f32)
        # identity for the output transposes: bf16, and its gpsimd emission
        # is deferred until the MLP phase (see below) so it never delays the
        # startup weight DMAs.
        ident_s = aux.tile([P, P], bf16)

        # x1 (LN1 output, bf16) outlives the attention/O-proj scopes below.
        # bf16 is used both as the MLP input and the residual-2 operand; the
        # ~0.4% rounding is far inside the tolerance budget.
        keep = ctx.enter_context(tc.tile_pool(name="keep", bufs=1))
        x1b_s = keep.tile([P, HC, SQ], bf16)
        # weight-stream pools live low in SBUF so their DMAs never alias the
        # attention-phase pools and can prefetch during earlier phases
        wop = ctx.enter_context(tc.tile_pool(name="wo_st", bufs=3))
        w1p = ctx.enter_context(tc.tile_pool(name="w1_st", bufs=3))

        def ln_rows(pool, sum_ps, sq_ps):
            """Turn the accumulated sum/sq-sum psum rows into partition-
            broadcast mean/rstd' tiles. The critical chain avoids both the
            mean dependency (var·H² = sq·H − sum²) and the slow DVE
            reciprocal (rstd' = exp(−½·ln(varH2+εH²)) on Act; ln and exp
            share an activation table). rstd' = rstd/H — the missing ×H is
            folded into the affine weight (see *_wH tiles)."""
            # mean first: the mb broadcast (GpSimd) overlaps the Act/DVE
            # variance chain, so the first t1 never waits on it
            m2r = pool.tile([1, SQ], f32, tag="lnsc", bufs=2)
            nc.scalar.activation(m2r[:], sum_ps[:], AF.Square)
            mean = pool.tile([1, SQ], bf16, tag="lnmean", bufs=1)
            nc.vector.tensor_scalar_mul(mean[:], sum_ps[:], 1.0 / H)
            mb = pool.tile([P, SQ], bf16, tag="lnmb", bufs=1)
            nc.gpsimd.partition_broadcast(mb[:], mean[:], channels=P)
            varh = pool.tile([1, SQ], f32, tag="lnsc", bufs=2)
            nc.vector.scalar_tensor_tensor(
                out=varh[:], in0=sq_ps[:], scalar=float(H), in1=m2r[:],
                op0=OP.mult, op1=OP.subtract,
            )
            lnv = pool.tile([1, SQ], f32, tag="lnsc", bufs=2)
            nc.scalar.activation(lnv[:], varh[:], AF.Ln, bias=epsh_s[:])
            rstd = pool.tile([1, SQ], bf16, tag="lnrstd", bufs=1)
            with nc.allow_low_precision(reason="act-table rstd is benign"):
                nc.scalar.activation(rstd[:], lnv[:], AF.Exp, scale=-0.5)
            rb = pool.tile([P, SQ], bf16, tag="lnrb", bufs=1)
            nc.gpsimd.partition_broadcast(rb[:], rstd[:], channels=P)
            return mb, rb

        def ln_chunks(pool, src, mb, rb, emit_chunk):
            """Per-chunk normalization: subtract on GpSimd, multiply on DVE
            (pipelined); `emit_chunk(j, t2)` emits the affine + stores."""
            for j in range(HC):
                t1 = pool.tile([P, SQ], bf16, tag="lnt1", bufs=2)
                nc.gpsimd.tensor_sub(t1[:], src[:, j, :], mb[:])
                t2 = pool.tile([P, SQ], bf16, tag="lnt2", bufs=2)
                nc.vector.tensor_tensor(t2[:], t1[:], rb[:], OP.mult)
                emit_chunk(j, t2)

        with tc.tile_pool(name="resid", bufs=1) as resid:
            # xT lives here (not in the QKV scope): columns 0:SQ are this
            # core's query tokens and double as the attention residual.
            xT_s = resid.tile([P, HC, S], bf16)
            xt_src = t["xT"].rearrange("(c p) s -> p c s", p=P)
            for j in range(HC):
                eng = nc.sync if j % 2 == 0 else nc.scalar
                eng.dma_start(xT_s[:, j, :], xt_src[:, j, :])
            flush_aux()
            with tc.tile_pool(name="attn_out", bufs=1) as aop:
                attnT_s = aop.tile([P, HC, SQ], bf16)

                with tc.tile_pool(name="qkv_keep", bufs=1) as p2:
                    # qTz[p, h, q]: head h's 64 q-rows live at partitions
                    # (h%2)*64..+64 of plane h; the other 64 partitions stay
                    # zero so scores can contract over all 128 partitions
                    # (full PE-array activity keeps the HAM clock warm).
                    qTz_s = p2.tile([P, NH, SQ], bf16)
                    kT_s = p2.tile([P, HC, S], bf16)
                    # v_s head-pair blocks of 161 columns:
                    #   [V_even(0:64) | ones_e(64) | ones_o(65) | 0(66:97) |
                    #    V_odd(97:161)]
                    # Even head 2j takes the 128-col stationary slice at
                    # 161j+0 (V rows -> psum partitions 0:64, denominator at
                    # 64); odd head 2j+1 takes the slice at 161j+33 (V rows
                    # -> partitions 64:128, denominator at partition 32 —
                    # engine partition accesses must start at multiples of
                    # 32). This keeps every head's attn rows partition-
                    # aligned with its attnT destination, so normalization
                    # never needs a partition-shifting DMA.
                    VB = 161
                    v_s = p2.tile([P, TS, VB * HC], bf16)
                    v_view = v_s[:].rearrange("p t (j c) -> p t j c", j=HC)

                    # ---------------- QKV projections ----------------
                    with tc.tile_pool(
                        name="wstream", bufs=3
                    ) as ws, tc.tile_pool(
                        name="qkv_ps", bufs=3, space="PSUM"
                    ) as pp:
                        # zero-fill memsets on the (otherwise idle) Vector
                        # engine, split per plane-half so the first Q bias
                        # write only waits for its own planes.
                        for j in range(HC):
                            nc.vector.memset(qTz_s[DH:P, 2 * j, :], 0.0)
                            nc.vector.memset(qTz_s[0:DH, 2 * j + 1, :], 0.0)
                        nc.vector.memset(v_view[:, :, :, DH : DH + 2], 1.0)
                        nc.vector.memset(v_view[:, :, :, DH + 2 : 97], 0.0)

                        # Q (our 1024 query tokens = xT columns 0:SQ)
                        for j in range(HC):
                            w_t = ws.tile([P, HC, P], bf16, tag="w")
                            nc.gpsimd.dma_start(
                                w_t[:],
                                t["Wq"][:, j * P : (j + 1) * P].rearrange(
                                    "(c p) m -> p c m", p=P
                                ),
                            )
                            ps = pp.tile([P, SQ], f32, tag="qkps")
                            for kc in range(HC):
                                for n in range(2):
                                    mm(
                                        ps[:, n * 512 : (n + 1) * 512],
                                        w_t[:, kc, :],
                                        xT_s[:, kc, n * 512 : (n + 1) * 512],
                                        kc == 0,
                                        kc == HC - 1,
                                    )
                            nc.scalar.activation(
                                qTz_s[0:DH, 2 * j, :], ps[0:DH, :],
                                AF.Identity, bias=bq_s[0:DH, j : j + 1],
                            )
                            nc.scalar.activation(
                                qTz_s[DH:P, 2 * j + 1, :], ps[DH:P, :],
                                AF.Identity, bias=bq_s[DH:P, j : j + 1],
                            )

                        # K (all 2048 tokens, bf16)
                        for j in range(HC):
                            wk_t = ws.tile([P, HC, P], bf16, tag="w")
                            nc.gpsimd.dma_start(
                                wk_t[:],
                                t["Wk"][:, j * P : (j + 1) * P].rearrange(
                                    "(c p) m -> p c m", p=P
                                ),
                            )
                            # kc outer over both sequence halves: one
                            # stationary load serves 4 matmuls
                            psk0 = pp.tile([P, SQ], f32, tag="qkps")
                            psk1 = pp.tile([P, SQ], f32, tag="qkps")
                            psk = [psk0, psk1]
                            for kc in range(HC):
                                for hf in range(2):
                                    for n in range(2):
                                        mm(
                                            psk[hf][:, n * 512 : (n + 1) * 512],
                                            wk_t[:, kc, :],
                                            xT_s[
                                                :, kc,
                                                hf * SQ + n * 512 :
                                                hf * SQ + (n + 1) * 512,
                                            ],
                                            kc == 0,
                                            kc == HC - 1,
                                        )
                            for hf in range(2):
                                nc.scalar.activation(
                                    kT_s[:, j, hf * SQ : (hf + 1) * SQ],
                                    psk[hf][:],
                                    AF.Identity,
                                    bias=bk_s[:, j : j + 1],
                                )

                        # V (token-major with per-head ones column)
                        wv_t = ws.tile([P, HC, H], bf16, tag="wv", bufs=1)
                        nc.gpsimd.dma_start(
                            wv_t[:], t["Wv"].rearrange("(c p) m -> p c m", p=P)
                        )
                        for tt in range(TS):
                            ps = pp.tile([P, SQ], f32, tag="qkps")
                            for kc in range(HC):
                                mm(
                                    ps[:, 0:512],
                                    xT_s[:, kc, tt * P : (tt + 1) * P],
                                    wv_t[:, kc, 0:512],
                                    kc == 0,
                                    kc == HC - 1,
                                )
                                mm(
                                    ps[:, 512:H],
                                    xT_s[:, kc, tt * P : (tt + 1) * P],
                                    wv_t[:, kc, 512:H],
                                    kc == 0,
                                    kc == HC - 1,
                                )
                            ps_v = ps[:, 0:H].rearrange(
                                "p (j two d) -> p j two d", j=HC, two=2
                            )
                            bv_v = bvb_s[:].rearrange(
                                "p (j two d) -> p j two d", j=HC, two=2
                            )
                            nc.vector.scalar_tensor_tensor(
                                out=v_view[:, tt, :, 0:DH],
                                in0=ps_v[:, :, 0, :],
                                scalar=1.0,
                                in1=bv_v[:, :, 0, :],
                                op0=OP.mult,
                                op1=OP.add,
                            )
                            nc.vector.scalar_tensor_tensor(
                                out=v_view[:, tt, :, 97:161],
                                in0=ps_v[:, :, 1, :],
                                scalar=1.0,
                                in1=bv_v[:, :, 1, :],
                                op0=OP.mult,
                                op1=OP.add,
                            )

                    # ---------------- attention ----------------
                    with tc.tile_pool(name="attn_sb", bufs=1) as ab, tc.tile_pool(
                        name="probs", bufs=4
                    ) as prp, tc.tile_pool(
                        name="sc_ps", bufs=2, space="PSUM"
                    ) as pps, tc.tile_pool(
                        name="av_ps", bufs=2, space="PSUM"
                    ) as ppa:
                        avs = {}
                        spills = {}
                        bcs = {}

                        def spill_head(h):
                            # Copy the raw accumulator (attn rows + sums row,
                            # already partition-aligned with attnT) to SBUF on
                            # the DVE, compute the reciprocal row, broadcast
                            # it on GpSimd. For the last two heads the
                            # reciprocal runs on the (by-then idle) Act engine
                            # as exp(−ln d) — ~2.4µs instead of the 7.8µs DVE
                            # reciprocal, shortening the attention tail.
                            av = avs.pop(h)
                            avs_sb = ab.tile([P, SQ], f32, tag="avsb", bufs=3)
                            if h % 2 == 0:
                                dlo, dhi = 0, DH
                                drow = DH
                            else:
                                dlo, dhi = DH, P
                                drow = 32
                            nc.vector.tensor_copy(
                                avs_sb[dlo:dhi, :], av[dlo:dhi, :]
                            )
                            spills[h] = avs_sb
                            # the denominator row is read straight from PSUM
                            rec = ab.tile([1, SQ], f32r, tag="rec", bufs=3)
                            with nc.allow_low_precision(
                                reason="softmax recip rounding is benign"
                            ):
                                if h >= NH - 1:
                                    lnd = ab.tile(
                                        [1, SQ], f32, tag="lnd", bufs=2
                                    )
                                    nc.scalar.activation(
                                        lnd[:], av[drow : drow + 1, :],
                                        AF.Ln,
                                    )
                                    nc.scalar.activation(
                                        rec[:], lnd[:], AF.Exp, scale=-1.0
                                    )
                                else:
                                    nc.vector.reciprocal(
                                        rec[:], av[drow : drow + 1, :]
                                    )
                            bc = ab.tile([P, SQ], f32r, tag="bcs", bufs=3)
                            nc.gpsimd.partition_broadcast(
                                bc[:], rec[:], channels=P
                            )
                            bcs[h] = bc

                        def normalize_head(h):
                            """Divide head h's attention rows by the softmax
                            sums and place them into attnT. Emitted two heads
                            behind the matmul stream, entirely off the PE,
                            partition-aligned for both parities."""
                            hc = h // 2
                            avs_sb = spills.pop(h)
                            bc = bcs.pop(h)
                            if h % 2 == 0:
                                nc.vector.tensor_tensor(
                                    attnT_s[0:DH, hc, :], avs_sb[0:DH, :],
                                    bc[0:DH, :], OP.mult,
                                )
                            else:
                                nc.vector.tensor_tensor(
                                    attnT_s[DH:P, hc, :], avs_sb[DH:P, :],
                                    bc[DH:P, :], OP.mult,
                                )

                        def emit_av(h, av, kt, pr):
                            base = VB * (h // 2) + (0 if h % 2 == 0 else 33)
                            for n in range(2):
                                mm(
                                    av[:, n * 512 : (n + 1) * 512],
                                    v_s[:, kt, base : base + P],
                                    pr[:, n * 512 : (n + 1) * 512],
                                    kt == 0,
                                    kt == TS - 1,
                                )

                        for h in range(NH):
                            hc = h // 2
                            av = ppa.tile([P, SQ], f32, tag="av")
                            avs[h] = av
                            pending = []
                            for kt in range(TS):
                                sc = pps.tile([P, SQ], f32, tag="sc")
                                lhsT_k = kT_s[
                                    :, hc, kt * P : (kt + 1) * P
                                ]
                                for n in range(2):
                                    mm(
                                        sc[:, n * 512 : (n + 1) * 512],
                                        lhsT_k,
                                        qTz_s[
                                            :, h, n * 512 : (n + 1) * 512
                                        ],
                                        True,
                                        True,
                                    )
                                pr = prp.tile([P, SQ], bf16, tag="pr")
                                nc.scalar.activation(
                                    pr[:], sc[:], AF.Exp, bias=zero_s[:],
                                    scale=0.125,
                                )
                                pending.append((kt, pr))
                                if len(pending) > 2:
                                    emit_av(h, av, *pending.pop(0))
                            for p_ in pending:
                                emit_av(h, av, *p_)
                            spill_head(h)
                            if h >= 2:
                                normalize_head(h - 2)
                        normalize_head(NH - 2)
                        normalize_head(NH - 1)

                # ------------- O-projection + residual + LN1 -------------
                with tc.tile_pool(name="oproj", bufs=1) as op_, tc.tile_pool(
                    name="o_ps", bufs=2, space="PSUM"
                ) as ppo, tc.tile_pool(
                    name="st_ps", bufs=1, space="PSUM"
                ) as ppst:
                    nc.vector.tensor_scalar_mul(
                        l1wH_s[:], l1w_s[:], float(H)
                    )
                    nc.vector.tensor_scalar_mul(
                        l2wH_s[:], l2w_s[:], float(H)
                    )
                    r1_s = op_.tile([P, HC, SQ], bf16)
                    sum_ps = ppst.tile([1, SQ], f32, tag="lnsum", bufs=1)
                    sq_ps = ppst.tile([1, SQ], f32, tag="lnsq", bufs=1)
                    for j in range(HC):
                        wo_t = wop.tile([P, HC, P], bf16, tag="wo")
                        nc.gpsimd.dma_start(
                            wo_t[:],
                            t["Wo"][:, j * P : (j + 1) * P].rearrange(
                                "(c p) m -> p c m", p=P
                            ),
                        )
                        ps = ppo.tile([P, SQ], f32, tag="ops")
                        for kc in range(HC):
                            for n in range(2):
                                mm(
                                    ps[:, n * 512 : (n + 1) * 512],
                                    wo_t[:, kc, :],
                                    attnT_s[
                                        :, kc, n * 512 : (n + 1) * 512
                                    ],
                                    kc == 0,
                                    kc == HC - 1,
                                )
                        nc.vector.scalar_tensor_tensor(
                            out=r1_s[:, j, :],
                            in0=ps[:],
                            scalar=bo_s[:, j : j + 1],
                            in1=xT_s[:, j, 0:SQ],
                            op0=OP.add,
                            op1=OP.add,
                        )
                        # LN1 stats accumulate as the chunks appear
                        sq_t = op_.tile([P, SQ], bf16, tag="lnsqt", bufs=2)
                        nc.vector.tensor_tensor(
                            sq_t[:], r1_s[:, j, :], r1_s[:, j, :], OP.mult
                        )
                        for n in range(2):
                            mm(
                                sum_ps[:, n * 512 : (n + 1) * 512],
                                ones_s[:],
                                r1_s[:, j, n * 512 : (n + 1) * 512],
                                j == 0,
                                j == HC - 1,
                            )
                            mm(
                                sq_ps[:, n * 512 : (n + 1) * 512],
                                ones_s[:],
                                sq_t[:, n * 512 : (n + 1) * 512],
                                j == 0,
                                j == HC - 1,
                            )
                    def ln1_chunk(j, t2):
                        nc.scalar.activation(
                            x1b_s[:, j, :], t2[:], AF.Identity,
                            scale=l1wH_s[:, j : j + 1],
                            bias=l1b_s[:, j : j + 1],
                        )

                    # prefetch the first W1 chunks now: the GpSimd queue is
                    # about to fill with LN1 work, and MLP1's first matmuls
                    # should only wait on x1b chunks, not weights
                    w1_pre = []
                    for m in range(3):
                        w1_t = w1p.tile([P, HC, P], bf16, tag="w1")
                        nc.gpsimd.dma_start(
                            w1_t[:],
                            t["W1"][:, m * P : (m + 1) * P].rearrange(
                                "(c p) n -> p c n", p=P
                            ),
                        )
                        w1_pre.append(w1_t)

                    mb1, rb1 = ln_rows(op_, sum_ps, sq_ps)
                    ln_chunks(op_, r1_s, mb1, rb1, ln1_chunk)

        # ---------------- MLP + LN2 + output ----------------
        with tc.tile_pool(name="mlp", bufs=1) as mp:
            hT_s = mp.tile([P, FC, SQ], bf16)
            r2_s = mp.tile([P, HC, SQ], bf16)
            w2_s = mp.tile([P, FC, H], bf16)
            # W2 prefetch: issue early on the idle Sync queue, chunked so
            # the first MLP2 matmul doesn't wait on the whole 4.7MB.
            w2_src = t["W2"].rearrange("(c p) m -> p c m", p=P)
            for ci in range(4):
                nc.sync.dma_start(
                    w2_s[:, ci * 6 : (ci + 1) * 6, :],
                    w2_src[:, ci * 6 : (ci + 1) * 6, :],
                )
            with tc.tile_pool(
                name="m_ps", bufs=2, space="PSUM"
            ) as ppm, tc.tile_pool(
                name="st2_ps", bufs=1, space="PSUM"
            ) as ppst2:
                for m in range(FC):
                    if m < len(w1_pre):
                        w1_t = w1_pre[m]
                    else:
                        w1_t = w1p.tile([P, HC, P], bf16, tag="w1")
                        nc.gpsimd.dma_start(
                            w1_t[:],
                            t["W1"][:, m * P : (m + 1) * P].rearrange(
                                "(c p) n -> p c n", p=P
                            ),
                        )
                    ps = ppm.tile([P, SQ], f32, tag="mps")
                    for kc in range(HC):
                        for n in range(2):
                            mm(
                                ps[:, n * 512 : (n + 1) * 512],
                                w1_t[:, kc, :],
                                x1b_s[:, kc, n * 512 : (n + 1) * 512],
                                kc == 0,
                                kc == HC - 1,
                            )
                    nc.scalar.activation(
                        hT_s[:, m, :], ps[:], AF.Gelu, bias=b1_s[:, m : m + 1]
                    )

                # identity for the output transposes (gpsimd is idle here)
                make_identity(nc, ident_s[:])

                sum2_ps = ppst2.tile([1, SQ], f32, tag="ln2sum", bufs=1)
                sq2_ps = ppst2.tile([1, SQ], f32, tag="ln2sq", bufs=1)
                for j in range(HC):
                    ps = ppm.tile([P, SQ], f32, tag="mps")
                    for kc in range(FC):
                        for n in range(2):
                            mm(
                                ps[:, n * 512 : (n + 1) * 512],
                                w2_s[:, kc, j * P : (j + 1) * P],
                                hT_s[:, kc, n * 512 : (n + 1) * 512],
                                kc == 0,
                                kc == FC - 1,
                            )
                    nc.vector.scalar_tensor_tensor(
                        out=r2_s[:, j, :],
                        in0=ps[:],
                        scalar=b2_s[:, j : j + 1],
                        in1=x1b_s[:, j, :],
                        op0=OP.add,
                        op1=OP.add,
                    )
                    sq_t = mp.tile([P, SQ], bf16, tag="ln2sqt", bufs=1)
                    nc.vector.tensor_tensor(
                        sq_t[:], r2_s[:, j, :], r2_s[:, j, :], OP.mult
                    )
                    for n in range(2):
                        mm(
                            sum2_ps[:, n * 512 : (n + 1) * 512],
                            ones_s[:],
                            r2_s[:, j, n * 512 : (n + 1) * 512],
                            j == 0,
                            j == HC - 1,
                        )
                        mm(
                            sq2_ps[:, n * 512 : (n + 1) * 512],
                            ones_s[:],
                            sq_t[:, n * 512 : (n + 1) * 512],
                            j == 0,
                            j == HC - 1,
                        )
                mb2, rb2 = ln_rows(mp, sum2_ps, sq2_ps)

            # LN2 chunks + transpose back to token-major + store, pipelined
            # per feature chunk (the MLP psum pools are closed here, freeing
            # banks for the transpose pool)
            with tc.tile_pool(name="outp", bufs=1) as outp, tc.tile_pool(
                name="tr_ps", bufs=4, space="PSUM"
            ) as ppt:

                def ln2_chunk(j, t2):
                    r2n = outp.tile([P, SQ], bf16, tag="r2n", bufs=2)
                    nc.vector.tensor_scalar(
                        r2n[:], t2[:], l2wH_s[:, j : j + 1],
                        l2b_s[:, j : j + 1], OP.mult, OP.add,
                    )
                    stage = outp.tile([P, TQ, P], f32, tag="out", bufs=2)
                    for tp in range(TQ // 2):
                        tps = ppt.tile([P, 2, P], bf16, tag="tr")
                        for k in range(2):
                            nc.tensor.transpose(
                                tps[:, k, :],
                                r2n[:, (2 * tp + k) * P : (2 * tp + k + 1) * P],
                                ident_s[:],
                            )
                        nc.scalar.activation(
                            stage[:, 2 * tp : 2 * tp + 2, :], tps[:],
                            AF.Identity, bias=zero_s[:],
                        )
                    eng = nc.sync if j % 2 == 0 else nc.scalar
                    eng.dma_start(
                        t["y"][:, j * P : (j + 1) * P].rearrange(
                            "(t p) f -> p t f", p=P
                        ),
                        stage[:],
                    )

                ln_chunks(mp, r2_s, mb2, rb2, ln2_chunk)


def _build():
    import concourse.bacc as bacc
    import concourse.tile as tile
    import concourse.mybir as mybir
    from concourse.masks import make_identity

    f32 = mybir.dt.float32
    f32r = mybir.dt.float32r
    bf16 = mybir.dt.bfloat16

    nc = bacc.Bacc(
        "TRN2", target_bir_lowering=False, debug=False, num_devices=N_CORES
    )
    specs = [
        ("xT", [H, S], bf16, "ExternalInput"),
        ("Wq", [H, H], bf16, "ExternalInput"),
        ("Wk", [H, H], bf16, "ExternalInput"),
        ("Wv", [H, H], bf16, "ExternalInput"),
        ("Wo", [H, H], bf16, "ExternalInput"),
        ("W1", [H, FF], bf16, "ExternalInput"),
        ("W2", [FF, H], bf16, "ExternalInput"),
        ("bq2", [P, HC], f32, "ExternalInput"),
        ("bk2", [P, HC], f32, "ExternalInput"),
        ("bv", [H], f32, "ExternalInput"),
        ("bo2", [P, HC], f32, "ExternalInput"),
        ("b12", [P, FC], f32, "ExternalInput"),
        ("b22", [P, HC], f32, "ExternalInput"),
        ("l1w", [P, HC], f32, "ExternalInput"),
        ("l1b", [P, HC], f32, "ExternalInput"),
        ("l2w", [P, HC], f32, "ExternalInput"),
        ("l2b", [P, HC], f32, "ExternalInput"),
        ("y", [SQ, H], f32, "ExternalOutput"),
    ]
    t = {
        name: nc.dram_tensor(name, shape, dt, kind=kind).ap()
        for name, shape, dt, kind in specs
    }
    with tile.TileContext(nc) as tc:
        _emit(nc, tc, t, mybir, make_identity)
    nc.compile()
    return nc


def _chunk_major(v):
    """[C*P] -> [P, C] with entry [p, c] = v[c*P + p]."""
    return np.ascontiguousarray(v.reshape(-1, P).T)


def prepare_in_maps(inputs):
    inp = {k: np.asarray(v) for k, v in inputs.items()}
    x = inp["x"].astype(np.float32)

    shared = {
        "Wq": inp["Wq"].astype(BF16),
        "Wk": inp["Wk"].astype(BF16),
        "Wv": inp["Wv"].astype(BF16),
        "Wo": inp["Wo"].astype(BF16),
        "W1": inp["W1"].astype(BF16),
        "W2": inp["W2"].astype(BF16),
        "bq2": _chunk_major(inp["bq"].astype(np.float32)),
        "bk2": _chunk_major(inp["bk"].astype(np.float32)),
        "bv": inp["bv"].astype(np.float32),
        "bo2": _chunk_major(inp["bo"].astype(np.float32)),
        "b12": _chunk_major(inp["b1"].astype(np.float32)),
        "b22": _chunk_major(inp["b2"].astype(np.float32)),
        "l1w": _chunk_major(inp["ln1_w"].astype(np.float32)),
        "l1b": _chunk_major(inp["ln1_b"].astype(np.float32)),
        "l2w": _chunk_major(inp["ln2_w"].astype(np.float32)),
        "l2b": _chunk_major(inp["ln2_b"].astype(np.float32)),
    }
    in_maps = []
    for c in range(N_CORES):
        b, hf = c // 2, c % 2
        xT = x[b].T
        if hf:
            # rotate so this core's query half sits at columns 0:SQ (the
            # program is SPMD-identical; key order is irrelevant since
            # attention reduces over all keys)
            xT = np.concatenate([xT[:, SQ:], xT[:, :SQ]], axis=1)
        m = dict(shared)
        m["xT"] = np.ascontiguousarray(xT).astype(BF16)
        in_maps.append(m)
    return in_maps


def get_program():
    if "nc" not in _CACHE:
        _CACHE["nc"] = _build()
    return _CACHE["nc"]


def kernel(**inputs):
    from concourse.bass_utils import run_bass_kernel_spmd

    nc = get_program()
    in_maps = prepare_in_maps(inputs)
    res = run_bass_kernel_spmd(nc, in_maps, core_ids=list(range(N_CORES)))
    out = np.empty((B, S, H), np.float32)
    for c in range(N_CORES):
        b, hf = c // 2, c % 2
        out[b, hf * SQ : (hf + 1) * SQ] = res.results[c]["y"]
    return out


# revision 59
# speedup vs baseline: 1.0027x; 1.0021x over previous
"""BertBlock kernel for 8 Trainium2 NeuronCores.

Sharding: pure data-parallel over (batch, half-sequence) tokens: core c
handles batch element c//2, query-token half c%2 (1024 tokens). Each core
recomputes K/V for the full 2048-token sequence of its batch element (the
duplicated K/V projection work is far cheaper than any 2-rank collective),
so no collectives are needed at all.

Device layout is feature-major ([feature, token]) end to end; the host
pre-transposes each core's x slice and rotates it so the core's own query
half always sits at columns 0:SQ (the program is SPMD-identical; key
order is irrelevant to the attention reduction). x is bf16 everywhere,
doubling as the attention residual. Softmax denominators come from
ones-columns in the attention-V stationary blocks, with even/odd heads
taking partition-aligned 128-col slices of a 161-col head-pair block so
normalization never crosses the partition-64 boundary. Per-head softmax
normalization is fully off the PE: the av accumulator is spilled by the
DVE, the reciprocal row computed on the DVE (Act exp(-ln d) for the last
head to shorten the tail), partition-broadcast on GpSimd, and the divide
deferred two heads so the PE never waits. LayerNorm stats matmuls are
interleaved into the producing loops; rstd comes from Act's
exp(-0.5*ln(var*H^2)) (ln/exp are table activations, far faster than the
DVE reciprocal) with the 1/H folded into pre-scaled affine weights; the
per-chunk normalize is split across GpSimd/DVE/Act. Large memsets live on
the Vector engine and weight-chunk DMA issues go first on GpSimd so the
PE starts within ~18us.
"""

import numpy as np
import ml_dtypes

P = 128
B = 4
S = 2048          # sequence length (keys)
SQ = 1024         # query tokens per core
H = 768
HC = H // P       # 6 feature chunks
NH = 12
DH = 64
FF = 3072
FC = FF // P      # 24
TS = S // P       # 16 key-token chunks
TQ = SQ // P      # 8 query-token chunks
N_CORES = 8
EPS = 1e-5
BF16 = ml_dtypes.bfloat16

_CACHE = {}


def _emit(nc, tc, t, mybir, make_identity):
    """Emit the per-core program. `t` maps tensor name -> DRAM AP."""
    from contextlib import ExitStack

    f32 = mybir.dt.float32
    f32r = mybir.dt.float32r
    bf16 = mybir.dt.bfloat16
    AF = mybir.ActivationFunctionType
    OP = mybir.AluOpType

    def mm(ps, lhsT, rhs, start, stop):
        nc.tensor.matmul(ps, lhsT=lhsT, rhs=rhs, start=start, stop=stop)

    with ExitStack() as ctx:
        aux = ctx.enter_context(tc.tile_pool(name="aux", bufs=1))

        # aux tiles are allocated up front but their (small) loads are
        # issued on the Sync queue after the xTq input DMAs: GpSimd must
        # stay free for the weight-chunk DMAs the PE waits on at startup.
        _aux_pending = []

        def aux_load(name, shape, dtype=f32):
            tl = aux.tile(shape, dtype, tag=name)
            _aux_pending.append((tl, t[name]))
            return tl

        def flush_aux():
            for tl, src in _aux_pending:
                nc.sync.dma_start(tl[:], src)
            _aux_pending.clear()

        bq_s = aux_load("bq2", [P, HC])
        bk_s = aux_load("bk2", [P, HC])
        bo_s = aux_load("bo2", [P, HC])
        b2_s = aux_load("b22", [P, HC])
        l1w_s = aux_load("l1w", [P, HC])
        l1b_s = aux_load("l1b", [P, HC])
        l2w_s = aux_load("l2w", [P, HC])
        l2b_s = aux_load("l2b", [P, HC])
        b1_s = aux_load("b12", [P, FC])
        bvb_s = aux.tile([P, H], f32)
        _aux_pending.append((bvb_s, t["bv"].partition_broadcast(P)))
        ones_f = aux.tile([P, 1], f32)
        nc.vector.memset(ones_f[:], 1.0)
        ones_s = aux.tile([P, 1], f32r)
        nc.vector.tensor_copy(ones_s[:], ones_f[:])
        zero_s = aux.tile([P, 1], f32)
        nc.vector.memset(zero_s[:], 0.0)
        epsh_s = aux.tile([1, 1], f32)
        nc.vector.memset(epsh_s[:], EPS * H * H)
        # LN affine weights pre-scaled by H (ln_rows computes rstd/H);
        # filled right before the O-projection so the startup vector queue
        # stays clear of aux-DMA dependencies.
        l1wH_s = aux.tile([P, HC], f32)
        l2wH_s = aux.tile([P, HC], f32)
        # identity for the output transposes: bf16, and its gpsimd emission
        # is deferred until the MLP phase (see below) so it never delays the
        # startup weight DMAs.
        ident_s = aux.tile([P, P], bf16)

        # x1 (LN1 output, bf16) outlives the attention/O-proj scopes below.
        # bf16 is used both as the MLP input and the residual-2 operand; the
        # ~0.4% rounding is far inside the tolerance budget.
        keep = ctx.enter_context(tc.tile_pool(name="keep", bufs=1))
        x1b_s = keep.tile([P, HC, SQ], bf16)
        # weight-stream pools live low in SBUF so their DMAs never alias the
        # attention-phase pools and can prefetch during earlier phases
        wop = ctx.enter_context(tc.tile_pool(name="wo_st", bufs=3))
        w1p = ctx.enter_context(tc.tile_pool(name="w1_st", bufs=3))

        def ln_rows(pool, sum_ps, sq_ps):
            """Turn the accumulated sum/sq-sum psum rows into partition-
            broadcast mean/rstd' tiles. The critical chain avoids both the
            mean dependency (var·H² = sq·H − sum²) and the slow DVE
            reciprocal (rstd' = exp(−½·ln(varH2+εH²)) on Act; ln and exp
            share an activation table). rstd' = rstd/H — the missing ×H is
            folded into the affine weight (see *_wH tiles)."""
            # mean first: the mb broadcast (GpSimd) overlaps the Act/DVE
            # variance chain, so the first t1 never waits on it
            m2r = pool.tile([1, SQ], f32, tag="lnsc", bufs=2)
            nc.scalar.activation(m2r[:], sum_ps[:], AF.Square)
            mean = pool.tile([1, SQ], f32r, tag="lnmean", bufs=1)
            nc.vector.tensor_scalar_mul(mean[:], sum_ps[:], 1.0 / H)
            mb = pool.tile([P, SQ], f32r, tag="lnmb", bufs=1)
            nc.gpsimd.partition_broadcast(mb[:], mean[:], channels=P)
            varh = pool.tile([1, SQ], f32, tag="lnsc", bufs=2)
            nc.vector.scalar_tensor_tensor(
                out=varh[:], in0=sq_ps[:], scalar=float(H), in1=m2r[:],
                op0=OP.mult, op1=OP.subtract,
            )
            lnv = pool.tile([1, SQ], f32, tag="lnsc", bufs=2)
            nc.scalar.activation(lnv[:], varh[:], AF.Ln, bias=epsh_s[:])
            rstd = pool.tile([1, SQ], f32r, tag="lnrstd", bufs=1)
            with nc.allow_low_precision(reason="act-table rstd is benign"):
                nc.scalar.activation(rstd[:], lnv[:], AF.Exp, scale=-0.5)
            rb = pool.tile([P, SQ], f32r, tag="lnrb", bufs=1)
            nc.gpsimd.partition_broadcast(rb[:], rstd[:], channels=P)
            return mb, rb

        def ln_chunks(pool, src, mb, rb, emit_chunk):
            """Per-chunk normalization: subtract on GpSimd, multiply on DVE
            (pipelined); `emit_chunk(j, t2)` emits the affine + stores."""
            for j in range(HC):
                t1 = pool.tile([P, SQ], f32, tag="lnt1", bufs=2)
                nc.gpsimd.tensor_sub(t1[:], src[:, j, :], mb[:])
                t2 = pool.tile([P, SQ], f32, tag="lnt2", bufs=2)
                nc.vector.tensor_tensor(t2[:], t1[:], rb[:], OP.mult)
                emit_chunk(j, t2)

        with tc.tile_pool(name="resid", bufs=1) as resid:
            # xT lives here (not in the QKV scope): columns 0:SQ are this
            # core's query tokens and double as the attention residual.
            xT_s = resid.tile([P, HC, S], bf16)
            xt_src = t["xT"].rearrange("(c p) s -> p c s", p=P)
            for j in range(HC):
                eng = nc.sync if j % 2 == 0 else nc.scalar
                eng.dma_start(xT_s[:, j, :], xt_src[:, j, :])
            flush_aux()
            with tc.tile_pool(name="attn_out", bufs=1) as aop:
                attnT_s = aop.tile([P, HC, SQ], bf16)

                with tc.tile_pool(name="qkv_keep", bufs=1) as p2:
                    # qTz[p, h, q]: head h's 64 q-rows live at partitions
                    # (h%2)*64..+64 of plane h; the other 64 partitions stay
                    # zero so scores can contract over all 128 partitions
                    # (full PE-array activity keeps the HAM clock warm).
                    qTz_s = p2.tile([P, NH, SQ], bf16)
                    kT_s = p2.tile([P, HC, S], bf16)
                    # v_s head-pair blocks of 161 columns:
                    #   [V_even(0:64) | ones_e(64) | ones_o(65) | 0(66:97) |
                    #    V_odd(97:161)]
                    # Even head 2j takes the 128-col stationary slice at
                    # 161j+0 (V rows -> psum partitions 0:64, denominator at
                    # 64); odd head 2j+1 takes the slice at 161j+33 (V rows
                    # -> partitions 64:128, denominator at partition 32 —
                    # engine partition accesses must start at multiples of
                    # 32). This keeps every head's attn rows partition-
                    # aligned with its attnT destination, so normalization
                    # never needs a partition-shifting DMA.
                    VB = 161
                    v_s = p2.tile([P, TS, VB * HC], bf16)
                    v_view = v_s[:].rearrange("p t (j c) -> p t j c", j=HC)

                    # ---------------- QKV projections ----------------
                    with tc.tile_pool(
                        name="wstream", bufs=3
                    ) as ws, tc.tile_pool(
                        name="qkv_ps", bufs=3, space="PSUM"
                    ) as pp:
                        # zero-fill memsets on the (otherwise idle) Vector
                        # engine, split per plane-half so the first Q bias
                        # write only waits for its own planes.
                        for j in range(HC):
                            nc.vector.memset(qTz_s[DH:P, 2 * j, :], 0.0)
                            nc.vector.memset(qTz_s[0:DH, 2 * j + 1, :], 0.0)
                        nc.vector.memset(v_view[:, :, :, DH : DH + 2], 1.0)
                        nc.vector.memset(v_view[:, :, :, DH + 2 : 97], 0.0)

                        # Q (our 1024 query tokens = xT columns 0:SQ)
                        for j in range(HC):
                            w_t = ws.tile([P, HC, P], bf16, tag="w")
                            nc.gpsimd.dma_start(
                                w_t[:],
                                t["Wq"][:, j * P : (j + 1) * P].rearrange(
                                    "(c p) m -> p c m", p=P
                                ),
                            )
                            ps = pp.tile([P, SQ], f32, tag="qkps")
                            for kc in range(HC):
                                for n in range(2):
                                    mm(
                                        ps[:, n * 512 : (n + 1) * 512],
                                        w_t[:, kc, :],
                                        xT_s[:, kc, n * 512 : (n + 1) * 512],
                                        kc == 0,
                                        kc == HC - 1,
                                    )
                            nc.scalar.activation(
                                qTz_s[0:DH, 2 * j, :], ps[0:DH, :],
                                AF.Identity, bias=bq_s[0:DH, j : j + 1],
                            )
                            nc.scalar.activation(
                                qTz_s[DH:P, 2 * j + 1, :], ps[DH:P, :],
                                AF.Identity, bias=bq_s[DH:P, j : j + 1],
                            )

                        # K (all 2048 tokens, bf16)
                        for j in range(HC):
                            wk_t = ws.tile([P, HC, P], bf16, tag="w")
                            nc.gpsimd.dma_start(
                                wk_t[:],
                                t["Wk"][:, j * P : (j + 1) * P].rearrange(
                                    "(c p) m -> p c m", p=P
                                ),
                            )
                            # kc outer over both sequence halves: one
                            # stationary load serves 4 matmuls
                            psk0 = pp.tile([P, SQ], f32, tag="qkps")
                            psk1 = pp.tile([P, SQ], f32, tag="qkps")
                            psk = [psk0, psk1]
                            for kc in range(HC):
                                for hf in range(2):
                                    for n in range(2):
                                        mm(
                                            psk[hf][:, n * 512 : (n + 1) * 512],
                                            wk_t[:, kc, :],
                                            xT_s[
                                                :, kc,
                                                hf * SQ + n * 512 :
                                                hf * SQ + (n + 1) * 512,
                                            ],
                                            kc == 0,
                                            kc == HC - 1,
                                        )
                            for hf in range(2):
                                nc.scalar.activation(
                                    kT_s[:, j, hf * SQ : (hf + 1) * SQ],
                                    psk[hf][:],
                                    AF.Identity,
                                    bias=bk_s[:, j : j + 1],
                                )

                        # V (token-major with per-head ones column)
                        wv_t = ws.tile([P, HC, H], bf16, tag="wv", bufs=1)
                        nc.gpsimd.dma_start(
                            wv_t[:], t["Wv"].rearrange("(c p) m -> p c m", p=P)
                        )
                        for tt in range(TS):
                            ps = pp.tile([P, SQ], f32, tag="qkps")
                            for kc in range(HC):
                                mm(
                                    ps[:, 0:512],
                                    xT_s[:, kc, tt * P : (tt + 1) * P],
                                    wv_t[:, kc, 0:512],
                                    kc == 0,
                                    kc == HC - 1,
                                )
                                mm(
                                    ps[:, 512:H],
                                    xT_s[:, kc, tt * P : (tt + 1) * P],
                                    wv_t[:, kc, 512:H],
                                    kc == 0,
                                    kc == HC - 1,
                                )
                            ps_v = ps[:, 0:H].rearrange(
                                "p (j two d) -> p j two d", j=HC, two=2
                            )
                            bv_v = bvb_s[:].rearrange(
                                "p (j two d) -> p j two d", j=HC, two=2
                            )
                            nc.vector.scalar_tensor_tensor(
                                out=v_view[:, tt, :, 0:DH],
                                in0=ps_v[:, :, 0, :],
                                scalar=1.0,
                                in1=bv_v[:, :, 0, :],
                                op0=OP.mult,
                                op1=OP.add,
                            )
                            nc.vector.scalar_tensor_tensor(
                                out=v_view[:, tt, :, 97:161],
                                in0=ps_v[:, :, 1, :],
                                scalar=1.0,
                                in1=bv_v[:, :, 1, :],
                                op0=OP.mult,
                                op1=OP.add,
                            )

                    # ---------------- attention ----------------
                    with tc.tile_pool(name="attn_sb", bufs=1) as ab, tc.tile_pool(
                        name="probs", bufs=4
                    ) as prp, tc.tile_pool(
                        name="sc_ps", bufs=2, space="PSUM"
                    ) as pps, tc.tile_pool(
                        name="av_ps", bufs=2, space="PSUM"
                    ) as ppa:
                        avs = {}
                        spills = {}
                        bcs = {}

                        def spill_head(h):
                            # Copy the raw accumulator (attn rows + sums row,
                            # already partition-aligned with attnT) to SBUF on
                            # the DVE, compute the reciprocal row, broadcast
                            # it on GpSimd. For the last two heads the
                            # reciprocal runs on the (by-then idle) Act engine
                            # as exp(−ln d) — ~2.4µs instead of the 7.8µs DVE
                            # reciprocal, shortening the attention tail.
                            av = avs.pop(h)
                            avs_sb = ab.tile([P, SQ], f32, tag="avsb", bufs=3)
                            if h % 2 == 0:
                                dlo, dhi = 0, DH
                                drow = DH
                            else:
                                dlo, dhi = DH, P
                                drow = 32
                            nc.vector.tensor_copy(
                                avs_sb[dlo:dhi, :], av[dlo:dhi, :]
                            )
                            spills[h] = avs_sb
                            # the denominator row is read straight from PSUM
                            rec = ab.tile([1, SQ], f32r, tag="rec", bufs=3)
                            with nc.allow_low_precision(
                                reason="softmax recip rounding is benign"
                            ):
                                if h >= NH - 1:
                                    lnd = ab.tile(
                                        [1, SQ], f32, tag="lnd", bufs=2
                                    )
                                    nc.scalar.activation(
                                        lnd[:], av[drow : drow + 1, :],
                                        AF.Ln,
                                    )
                                    nc.scalar.activation(
                                        rec[:], lnd[:], AF.Exp, scale=-1.0
                                    )
                                else:
                                    nc.vector.reciprocal(
                                        rec[:], av[drow : drow + 1, :]
                                    )
                            bc = ab.tile([P, SQ], f32r, tag="bcs", bufs=3)
                            nc.gpsimd.partition_broadcast(
                                bc[:], rec[:], channels=P
                            )
                            bcs[h] = bc

                        def normalize_head(h):
                            """Divide head h's attention rows by the softmax
                            sums and place them into attnT. Emitted two heads
                            behind the matmul stream, entirely off the PE,
                            partition-aligned for both parities."""
                            hc = h // 2
                            avs_sb = spills.pop(h)
                            bc = bcs.pop(h)
                            if h % 2 == 0:
                                nc.vector.tensor_tensor(
                                    attnT_s[0:DH, hc, :], avs_sb[0:DH, :],
                                    bc[0:DH, :], OP.mult,
                                )
                            else:
                                nc.vector.tensor_tensor(
                                    attnT_s[DH:P, hc, :], avs_sb[DH:P, :],
                                    bc[DH:P, :], OP.mult,
                                )

                        def emit_av(h, av, kt, pr):
                            base = VB * (h // 2) + (0 if h % 2 == 0 else 33)
                            for n in range(2):
                                mm(
                                    av[:, n * 512 : (n + 1) * 512],
                                    v_s[:, kt, base : base + P],
                                    pr[:, n * 512 : (n + 1) * 512],
                                    kt == 0,
                                    kt == TS - 1,
                                )

                        for h in range(NH):
                            hc = h // 2
                            av = ppa.tile([P, SQ], f32, tag="av")
                            avs[h] = av
                            pending = []
                            for kt in range(TS):
                                sc = pps.tile([P, SQ], f32, tag="sc")
                                lhsT_k = kT_s[
                                    :, hc, kt * P : (kt + 1) * P
                                ]
                                for n in range(2):
                                    mm(
                                        sc[:, n * 512 : (n + 1) * 512],
                                        lhsT_k,
                                        qTz_s[
                                            :, h, n * 512 : (n + 1) * 512
                                        ],
                                        True,
                                        True,
                                    )
                                pr = prp.tile([P, SQ], bf16, tag="pr")
                                nc.scalar.activation(
                                    pr[:], sc[:], AF.Exp, bias=zero_s[:],
                                    scale=0.125,
                                )
                                pending.append((kt, pr))
                                if len(pending) > 2:
                                    emit_av(h, av, *pending.pop(0))
                            for p_ in pending:
                                emit_av(h, av, *p_)
                            spill_head(h)
                            if h >= 2:
                                normalize_head(h - 2)
                        normalize_head(NH - 2)
                        normalize_head(NH - 1)

                # ------------- O-projection + residual + LN1 -------------
                with tc.tile_pool(name="oproj", bufs=1) as op_, tc.tile_pool(
                    name="o_ps", bufs=2, space="PSUM"
                ) as ppo, tc.tile_pool(
                    name="st_ps", bufs=1, space="PSUM"
                ) as ppst:
                    nc.vector.tensor_scalar_mul(
                        l1wH_s[:], l1w_s[:], float(H)
                    )
                    nc.vector.tensor_scalar_mul(
                        l2wH_s[:], l2w_s[:], float(H)
                    )
                    r1_s = op_.tile([P, HC, SQ], f32r)
                    sum_ps = ppst.tile([1, SQ], f32, tag="lnsum", bufs=1)
                    sq_ps = ppst.tile([1, SQ], f32, tag="lnsq", bufs=1)
                    for j in range(HC):
                        wo_t = wop.tile([P, HC, P], bf16, tag="wo")
                        nc.gpsimd.dma_start(
                            wo_t[:],
                            t["Wo"][:, j * P : (j + 1) * P].rearrange(
                                "(c p) m -> p c m", p=P
                            ),
                        )
                        ps = ppo.tile([P, SQ], f32, tag="ops")
                        for kc in range(HC):
                            for n in range(2):
                                mm(
                                    ps[:, n * 512 : (n + 1) * 512],
                                    wo_t[:, kc, :],
                                    attnT_s[
                                        :, kc, n * 512 : (n + 1) * 512
                                    ],
                                    kc == 0,
                                    kc == HC - 1,
                                )
                        nc.vector.scalar_tensor_tensor(
                            out=r1_s[:, j, :],
                            in0=ps[:],
                            scalar=bo_s[:, j : j + 1],
                            in1=xT_s[:, j, 0:SQ],
                            op0=OP.add,
                            op1=OP.add,
                        )
                        # LN1 stats accumulate as the chunks appear
                        sq_t = op_.tile([P, SQ], f32r, tag="lnsqt", bufs=2)
                        nc.vector.tensor_tensor(
                            sq_t[:], r1_s[:, j, :], r1_s[:, j, :], OP.mult
                        )
                        for n in range(2):
                            mm(
                                sum_ps[:, n * 512 : (n + 1) * 512],
                                ones_s[:],
                                r1_s[:, j, n * 512 : (n + 1) * 512],
                                j == 0,
                                j == HC - 1,
                            )
                            mm(
                                sq_ps[:, n * 512 : (n + 1) * 512],
                                ones_s[:],
                                sq_t[:, n * 512 : (n + 1) * 512],
                                j == 0,
                                j == HC - 1,
                            )
                    def ln1_chunk(j, t2):
                        nc.scalar.activation(
                            x1b_s[:, j, :], t2[:], AF.Identity,
                            scale=l1wH_s[:, j : j + 1],
                            bias=l1b_s[:, j : j + 1],
                        )

                    # prefetch the first W1 chunks now: the GpSimd queue is
                    # about to fill with LN1 work, and MLP1's first matmuls
                    # should only wait on x1b chunks, not weights
                    w1_pre = []
                    for m in range(3):
                        w1_t = w1p.tile([P, HC, P], bf16, tag="w1")
                        nc.gpsimd.dma_start(
                            w1_t[:],
                            t["W1"][:, m * P : (m + 1) * P].rearrange(
                                "(c p) n -> p c n", p=P
                            ),
                        )
                        w1_pre.append(w1_t)

                    mb1, rb1 = ln_rows(op_, sum_ps, sq_ps)
                    ln_chunks(op_, r1_s, mb1, rb1, ln1_chunk)

        # ---------------- MLP + LN2 + output ----------------
        with tc.tile_pool(name="mlp", bufs=1) as mp:
            hT_s = mp.tile([P, FC, SQ], bf16)
            r2_s = mp.tile([P, HC, SQ], f32r)
            w2_s = mp.tile([P, FC, H], bf16)
            # W2 prefetch: issue early on the idle Sync queue, chunked so
            # the first MLP2 matmul doesn't wait on the whole 4.7MB.
            w2_src = t["W2"].rearrange("(c p) m -> p c m", p=P)
            for ci in range(4):
                nc.sync.dma_start(
                    w2_s[:, ci * 6 : (ci + 1) * 6, :],
                    w2_src[:, ci * 6 : (ci + 1) * 6, :],
                )
            with tc.tile_pool(
                name="m_ps", bufs=2, space="PSUM"
            ) as ppm, tc.tile_pool(
                name="st2_ps", bufs=1, space="PSUM"
            ) as ppst2:
                for m in range(FC):
                    if m < len(w1_pre):
                        w1_t = w1_pre[m]
                    else:
                        w1_t = w1p.tile([P, HC, P], bf16, tag="w1")
                        nc.gpsimd.dma_start(
                            w1_t[:],
                            t["W1"][:, m * P : (m + 1) * P].rearrange(
                                "(c p) n -> p c n", p=P
                            ),
                        )
                    ps = ppm.tile([P, SQ], f32, tag="mps")
                    for kc in range(HC):
                        for n in range(2):
                            mm(
                                ps[:, n * 512 : (n + 1) * 512],
                                w1_t[:, kc, :],
                                x1b_s[:, kc, n * 512 : (n + 1) * 512],
                                kc == 0,
                                kc == HC - 1,
                            )
                    nc.scalar.activation(
                        hT_s[:, m, :], ps[:], AF.Gelu, bias=b1_s[:, m : m + 1]
                    )

                # identity for the output transposes (gpsimd is idle here)
                make_identity(nc, ident_s[:])

                sum2_ps = ppst2.tile([1, SQ], f32, tag="ln2sum", bufs=1)
                sq2_ps = ppst2.tile([1, SQ], f32, tag="ln2sq", bufs=1)
                for j in range(HC):
                    ps = ppm.tile([P, SQ], f32, tag="mps")
                    for kc in range(FC):
                        for n in range(2):
                            mm(
                                ps[:, n * 512 : (n + 1) * 512],
                                w2_s[:, kc, j * P : (j + 1) * P],
                                hT_s[:, kc, n * 512 : (n + 1) * 512],
                                kc == 0,
                                kc == FC - 1,
                            )
                    nc.vector.scalar_tensor_tensor(
                        out=r2_s[:, j, :],
                        in0=ps[:],
                        scalar=b2_s[:, j : j + 1],
                        in1=x1b_s[:, j, :],
                        op0=OP.add,
                        op1=OP.add,
                    )
                    sq_t = mp.tile([P, SQ], f32r, tag="ln2sqt", bufs=1)
                    nc.vector.tensor_tensor(
                        sq_t[:], r2_s[:, j, :], r2_s[:, j, :], OP.mult
                    )
                    for n in range(2):
                        mm(
                            sum2_ps[:, n * 512 : (n + 1) * 512],
                            ones_s[:],
                            r2_s[:, j, n * 512 : (n + 1) * 512],
                            j == 0,
                            j == HC - 1,
                        )
                        mm(
                            sq2_ps[:, n * 512 : (n + 1) * 512],
                            ones_s[:],
                            sq_t[:, n * 512 : (n + 1) * 512],
                            j == 0,
                            j == HC - 1,
                        )
                mb2, rb2 = ln_rows(mp, sum2_ps, sq2_ps)

            # LN2 chunks + transpose back to token-major + store, pipelined
            # per feature chunk (the MLP psum pools are closed here, freeing
            # banks for the transpose pool)
            with tc.tile_pool(name="outp", bufs=1) as outp, tc.tile_pool(
                name="tr_ps", bufs=4, space="PSUM"
            ) as ppt:

                def ln2_chunk(j, t2):
                    r2n = outp.tile([P, SQ], bf16, tag="r2n", bufs=2)
                    nc.vector.tensor_scalar(
                        r2n[:], t2[:], l2wH_s[:, j : j + 1],
                        l2b_s[:, j : j + 1], OP.mult, OP.add,
                    )
                    stage = outp.tile([P, TQ, P], f32, tag="out", bufs=2)
                    for tp in range(TQ // 2):
                        tps = ppt.tile([P, 2, P], bf16, tag="tr")
                        for k in range(2):
                            nc.tensor.transpose(
                                tps[:, k, :],
                                r2n[:, (2 * tp + k) * P : (2 * tp + k + 1) * P],
                                ident_s[:],
                            )
                        nc.scalar.activation(
                            stage[:, 2 * tp : 2 * tp + 2, :], tps[:],
                            AF.Identity, bias=zero_s[:],
                        )
                    eng = nc.sync if j % 2 == 0 else nc.scalar
                    eng.dma_start(
                        t["y"][:, j * P : (j + 1) * P].rearrange(
                            "(t p) f -> p t f", p=P
                        ),
                        stage[:],
                    )

                ln_chunks(mp, r2_s, mb2, rb2, ln2_chunk)


def _build():
    import concourse.bacc as bacc
    import concourse.tile as tile
    import concourse.mybir as mybir
    from concourse.masks import make_identity

    f32 = mybir.dt.float32
    f32r = mybir.dt.float32r
    bf16 = mybir.dt.bfloat16

    nc = bacc.Bacc(
        "TRN2", target_bir_lowering=False, debug=False, num_devices=N_CORES
    )
    specs = [
        ("xT", [H, S], bf16, "ExternalInput"),
        ("Wq", [H, H], bf16, "ExternalInput"),
        ("Wk", [H, H], bf16, "ExternalInput"),
        ("Wv", [H, H], bf16, "ExternalInput"),
        ("Wo", [H, H], bf16, "ExternalInput"),
        ("W1", [H, FF], bf16, "ExternalInput"),
        ("W2", [FF, H], bf16, "ExternalInput"),
        ("bq2", [P, HC], f32, "ExternalInput"),
        ("bk2", [P, HC], f32, "ExternalInput"),
        ("bv", [H], f32, "ExternalInput"),
        ("bo2", [P, HC], f32, "ExternalInput"),
        ("b12", [P, FC], f32, "ExternalInput"),
        ("b22", [P, HC], f32, "ExternalInput"),
        ("l1w", [P, HC], f32, "ExternalInput"),
        ("l1b", [P, HC], f32, "ExternalInput"),
        ("l2w", [P, HC], f32, "ExternalInput"),
        ("l2b", [P, HC], f32, "ExternalInput"),
        ("y", [SQ, H], f32, "ExternalOutput"),
    ]
    t = {
        name: nc.dram_tensor(name, shape, dt, kind=kind).ap()
        for name, shape, dt, kind in specs
    }
    with tile.TileContext(nc) as tc:
        _emit(nc, tc, t, mybir, make_identity)
    nc.compile()
    return nc


def _chunk_major(v):
    """[C*P] -> [P, C] with entry [p, c] = v[c*P + p]."""
    return np.ascontiguousarray(v.reshape(-1, P).T)


def prepare_in_maps(inputs):
    inp = {k: np.asarray(v) for k, v in inputs.items()}
    x = inp["x"].astype(np.float32)

    shared = {
        "Wq": inp["Wq"].astype(BF16),
        "Wk": inp["Wk"].astype(BF16),
        "Wv": inp["Wv"].astype(BF16),
        "Wo": inp["Wo"].astype(BF16),
        "W1": inp["W1"].astype(BF16),
        "W2": inp["W2"].astype(BF16),
        "bq2": _chunk_major(inp["bq"].astype(np.float32)),
        "bk2": _chunk_major(inp["bk"].astype(np.float32)),
        "bv": inp["bv"].astype(np.float32),
        "bo2": _chunk_major(inp["bo"].astype(np.float32)),
        "b12": _chunk_major(inp["b1"].astype(np.float32)),
        "b22": _chunk_major(inp["b2"].astype(np.float32)),
        "l1w": _chunk_major(inp["ln1_w"].astype(np.float32)),
        "l1b": _chunk_major(inp["ln1_b"].astype(np.float32)),
        "l2w": _chunk_major(inp["ln2_w"].astype(np.float32)),
        "l2b": _chunk_major(inp["ln2_b"].astype(np.float32)),
    }
    in_maps = []
    for c in range(N_CORES):
        b, hf = c // 2, c % 2
        xT = x[b].T
        if hf:
            # rotate so this core's query half sits at columns 0:SQ (the
            # program is SPMD-identical; key order is irrelevant since
            # attention reduces over all keys)
            xT = np.concatenate([xT[:, SQ:], xT[:, :SQ]], axis=1)
        m = dict(shared)
        m["xT"] = np.ascontiguousarray(xT).astype(BF16)
        in_maps.append(m)
    return in_maps


def get_program():
    if "nc" not in _CACHE:
        _CACHE["nc"] = _build()
    return _CACHE["nc"]


def kernel(**inputs):
    from concourse.bass_utils import run_bass_kernel_spmd

    nc = get_program()
    in_maps = prepare_in_maps(inputs)
    res = run_bass_kernel_spmd(nc, in_maps, core_ids=list(range(N_CORES)))
    out = np.empty((B, S, H), np.float32)
    for c in range(N_CORES):
        b, hf = c // 2, c % 2
        out[b, hf * SQ : (hf + 1) * SQ] = res.results[c]["y"]
    return out


# revision 60
# speedup vs baseline: 1.0946x; 1.0917x over previous
"""BertBlock kernel for 8 Trainium2 NeuronCores.

Sharding: pure data-parallel over (batch, half-sequence) tokens: core c
handles batch element c//2, query-token half c%2 (1024 tokens). Each core
recomputes K/V for the full 2048-token sequence of its batch element (the
duplicated K/V projection work is far cheaper than any 2-rank collective),
so no collectives are needed at all.

Device layout is feature-major ([feature, token]) end to end; the host
pre-transposes each core's x slice and rotates it so the core's own query
half always sits at columns 0:SQ (the program is SPMD-identical; key
order is irrelevant to the attention reduction). x is bf16 everywhere,
doubling as the attention residual. Softmax denominators come from
ones-columns in the attention-V stationary blocks, with even/odd heads
taking partition-aligned 128-col slices of a 161-col head-pair block so
normalization never crosses the partition-64 boundary. Per-head softmax
normalization is fully off the PE: the av accumulator is spilled by the
DVE, the reciprocal row computed on the DVE (Act exp(-ln d) for the last
head to shorten the tail), partition-broadcast on GpSimd, and the divide
deferred two heads so the PE never waits. LayerNorm stats matmuls are
interleaved into the producing loops; rstd comes from Act's
exp(-0.5*ln(var*H^2)) (ln/exp are table activations, far faster than the
DVE reciprocal) with the 1/H folded into pre-scaled affine weights; the
per-chunk normalize is split across GpSimd/DVE/Act. Large memsets live on
the Vector engine and weight-chunk DMA issues go first on GpSimd so the
PE starts within ~18us.
"""

import numpy as np
import ml_dtypes

P = 128
B = 4
S = 2048          # sequence length (keys)
SQ = 1024         # query tokens per core
H = 768
HC = H // P       # 6 feature chunks
NH = 12
DH = 64
FF = 3072
FC = FF // P      # 24
TS = S // P       # 16 key-token chunks
TQ = SQ // P      # 8 query-token chunks
N_CORES = 8
EPS = 1e-5
BF16 = ml_dtypes.bfloat16

_CACHE = {}


def _emit(nc, tc, t, mybir, make_identity):
    """Emit the per-core program. `t` maps tensor name -> DRAM AP."""
    from contextlib import ExitStack

    f32 = mybir.dt.float32
    f32r = mybir.dt.float32r
    bf16 = mybir.dt.bfloat16
    AF = mybir.ActivationFunctionType
    OP = mybir.AluOpType

    def mm(ps, lhsT, rhs, start, stop):
        nc.tensor.matmul(ps, lhsT=lhsT, rhs=rhs, start=start, stop=stop)

    with ExitStack() as ctx:
        aux = ctx.enter_context(tc.tile_pool(name="aux", bufs=1))

        # aux tiles are allocated up front but their (small) loads are
        # issued on the Sync queue after the xTq input DMAs: GpSimd must
        # stay free for the weight-chunk DMAs the PE waits on at startup.
        _aux_pending = []

        def aux_load(name, shape, dtype=f32):
            tl = aux.tile(shape, dtype, tag=name)
            _aux_pending.append((tl, t[name]))
            return tl

        def flush_aux():
            for tl, src in _aux_pending:
                nc.sync.dma_start(tl[:], src)
            _aux_pending.clear()

        bq_s = aux_load("bq2", [P, HC])
        bk_s = aux_load("bk2", [P, HC])
        bo_s = aux_load("bo2", [P, HC])
        b2_s = aux_load("b22", [P, HC])
        l1w_s = aux_load("l1w", [P, HC])
        l1b_s = aux_load("l1b", [P, HC])
        l2w_s = aux_load("l2w", [P, HC])
        l2b_s = aux_load("l2b", [P, HC])
        b1_s = aux_load("b12", [P, FC])
        bvb_s = aux.tile([P, H], f32)
        _aux_pending.append((bvb_s, t["bv"].partition_broadcast(P)))
        ones_s = aux.tile([P, 1], bf16)
        nc.vector.memset(ones_s[:], 1.0)
        zero_s = aux.tile([P, 1], f32)
        nc.vector.memset(zero_s[:], 0.0)
        epsh_s = aux.tile([1, 1], f32)
        nc.vector.memset(epsh_s[:], EPS * H * H)
        # LN affine weights pre-scaled by H (ln_rows computes rstd/H);
        # filled right before the O-projection so the startup vector queue
        # stays clear of aux-DMA dependencies.
        l1wH_s = aux.tile([P, HC], f32)
        l2wH_s = aux.tile([P, HC], f32)
        # identity for the output transposes: bf16, and its gpsimd emission
        # is deferred until the MLP phase (see below) so it never delays the
        # startup weight DMAs.
        ident_s = aux.tile([P, P], bf16)

        # x1 (LN1 output, bf16) outlives the attention/O-proj scopes below.
        # bf16 is used both as the MLP input and the residual-2 operand; the
        # ~0.4% rounding is far inside the tolerance budget.
        keep = ctx.enter_context(tc.tile_pool(name="keep", bufs=1))
        x1b_s = keep.tile([P, HC, SQ], bf16)
        # weight-stream pools live low in SBUF so their DMAs never alias the
        # attention-phase pools and can prefetch during earlier phases
        wop = ctx.enter_context(tc.tile_pool(name="wo_st", bufs=3))
        w1p = ctx.enter_context(tc.tile_pool(name="w1_st", bufs=3))

        def ln_rows(pool, sum_ps, sq_ps):
            """Turn the accumulated sum/sq-sum psum rows into partition-
            broadcast mean/rstd' tiles. The critical chain avoids both the
            mean dependency (var·H² = sq·H − sum²) and the slow DVE
            reciprocal (rstd' = exp(−½·ln(varH2+εH²)) on Act; ln and exp
            share an activation table). rstd' = rstd/H — the missing ×H is
            folded into the affine weight (see *_wH tiles)."""
            # mean first: the mb broadcast (GpSimd) overlaps the Act/DVE
            # variance chain, so the first t1 never waits on it
            m2r = pool.tile([1, SQ], f32, tag="lnsc", bufs=2)
            nc.scalar.activation(m2r[:], sum_ps[:], AF.Square)
            mean = pool.tile([1, SQ], bf16, tag="lnmean", bufs=1)
            nc.vector.tensor_scalar_mul(mean[:], sum_ps[:], 1.0 / H)
            mb = pool.tile([P, SQ], bf16, tag="lnmb", bufs=1)
            nc.gpsimd.partition_broadcast(mb[:], mean[:], channels=P)
            varh = pool.tile([1, SQ], f32, tag="lnsc", bufs=2)
            nc.vector.scalar_tensor_tensor(
                out=varh[:], in0=sq_ps[:], scalar=float(H), in1=m2r[:],
                op0=OP.mult, op1=OP.subtract,
            )
            lnv = pool.tile([1, SQ], f32, tag="lnsc", bufs=2)
            nc.scalar.activation(lnv[:], varh[:], AF.Ln, bias=epsh_s[:])
            rstd = pool.tile([1, SQ], bf16, tag="lnrstd", bufs=1)
            with nc.allow_low_precision(reason="act-table rstd is benign"):
                nc.scalar.activation(rstd[:], lnv[:], AF.Exp, scale=-0.5)
            rb = pool.tile([P, SQ], bf16, tag="lnrb", bufs=1)
            nc.gpsimd.partition_broadcast(rb[:], rstd[:], channels=P)
            return mb, rb

        def ln_chunks(pool, src, mb, rb, emit_chunk):
            """Per-chunk normalization: both ops on the DVE in bf16 (16-bit
            operands run at 2x); `emit_chunk(j, t2)` emits the affine."""
            for j in range(HC):
                t1 = pool.tile([P, SQ], bf16, tag="lnt1", bufs=2)
                nc.vector.tensor_tensor(t1[:], src[:, j, :], mb[:], OP.subtract)
                t2 = pool.tile([P, SQ], bf16, tag="lnt2", bufs=2)
                nc.vector.tensor_tensor(t2[:], t1[:], rb[:], OP.mult)
                emit_chunk(j, t2)

        with tc.tile_pool(name="resid", bufs=1) as resid:
            # xT lives here (not in the QKV scope): columns 0:SQ are this
            # core's query tokens and double as the attention residual.
            xT_s = resid.tile([P, HC, S], bf16)
            xt_src = t["xT"].rearrange("(c p) s -> p c s", p=P)
            for j in range(HC):
                eng = nc.sync if j % 2 == 0 else nc.scalar
                eng.dma_start(xT_s[:, j, :], xt_src[:, j, :])
            flush_aux()
            with tc.tile_pool(name="attn_out", bufs=1) as aop:
                attnT_s = aop.tile([P, HC, SQ], bf16)

                with tc.tile_pool(name="qkv_keep", bufs=1) as p2:
                    # qTz[p, h, q]: head h's 64 q-rows live at partitions
                    # (h%2)*64..+64 of plane h; the other 64 partitions stay
                    # zero so scores can contract over all 128 partitions
                    # (full PE-array activity keeps the HAM clock warm).
                    qTz_s = p2.tile([P, NH, SQ], bf16)
                    kT_s = p2.tile([P, HC, S], bf16)
                    # v_s head-pair blocks of 161 columns:
                    #   [V_even(0:64) | ones_e(64) | ones_o(65) | 0(66:97) |
                    #    V_odd(97:161)]
                    # Even head 2j takes the 128-col stationary slice at
                    # 161j+0 (V rows -> psum partitions 0:64, denominator at
                    # 64); odd head 2j+1 takes the slice at 161j+33 (V rows
                    # -> partitions 64:128, denominator at partition 32 —
                    # engine partition accesses must start at multiples of
                    # 32). This keeps every head's attn rows partition-
                    # aligned with its attnT destination, so normalization
                    # never needs a partition-shifting DMA.
                    VB = 161
                    v_s = p2.tile([P, TS, VB * HC], bf16)
                    v_view = v_s[:].rearrange("p t (j c) -> p t j c", j=HC)

                    # ---------------- QKV projections ----------------
                    with tc.tile_pool(
                        name="wstream", bufs=3
                    ) as ws, tc.tile_pool(
                        name="qkv_ps", bufs=3, space="PSUM"
                    ) as pp:
                        # zero-fill memsets on the (otherwise idle) Vector
                        # engine, split per plane-half so the first Q bias
                        # write only waits for its own planes.
                        for j in range(HC):
                            nc.vector.memset(qTz_s[DH:P, 2 * j, :], 0.0)
                            nc.vector.memset(qTz_s[0:DH, 2 * j + 1, :], 0.0)
                        nc.vector.memset(v_view[:, :, :, DH : DH + 2], 1.0)
                        nc.vector.memset(v_view[:, :, :, DH + 2 : 97], 0.0)

                        # Q (our 1024 query tokens = xT columns 0:SQ)
                        for j in range(HC):
                            w_t = ws.tile([P, HC, P], bf16, tag="w")
                            nc.gpsimd.dma_start(
                                w_t[:],
                                t["Wq"][:, j * P : (j + 1) * P].rearrange(
                                    "(c p) m -> p c m", p=P
                                ),
                            )
                            ps = pp.tile([P, SQ], f32, tag="qkps")
                            for kc in range(HC):
                                for n in range(2):
                                    mm(
                                        ps[:, n * 512 : (n + 1) * 512],
                                        w_t[:, kc, :],
                                        xT_s[:, kc, n * 512 : (n + 1) * 512],
                                        kc == 0,
                                        kc == HC - 1,
                                    )
                            nc.scalar.activation(
                                qTz_s[0:DH, 2 * j, :], ps[0:DH, :],
                                AF.Identity, bias=bq_s[0:DH, j : j + 1],
                            )
                            nc.scalar.activation(
                                qTz_s[DH:P, 2 * j + 1, :], ps[DH:P, :],
                                AF.Identity, bias=bq_s[DH:P, j : j + 1],
                            )

                        # K (all 2048 tokens, bf16)
                        for j in range(HC):
                            wk_t = ws.tile([P, HC, P], bf16, tag="w")
                            nc.gpsimd.dma_start(
                                wk_t[:],
                                t["Wk"][:, j * P : (j + 1) * P].rearrange(
                                    "(c p) m -> p c m", p=P
                                ),
                            )
                            # kc outer over both sequence halves: one
                            # stationary load serves 4 matmuls
                            psk0 = pp.tile([P, SQ], f32, tag="qkps")
                            psk1 = pp.tile([P, SQ], f32, tag="qkps")
                            psk = [psk0, psk1]
                            for kc in range(HC):
                                for hf in range(2):
                                    for n in range(2):
                                        mm(
                                            psk[hf][:, n * 512 : (n + 1) * 512],
                                            wk_t[:, kc, :],
                                            xT_s[
                                                :, kc,
                                                hf * SQ + n * 512 :
                                                hf * SQ + (n + 1) * 512,
                                            ],
                                            kc == 0,
                                            kc == HC - 1,
                                        )
                            for hf in range(2):
                                nc.scalar.activation(
                                    kT_s[:, j, hf * SQ : (hf + 1) * SQ],
                                    psk[hf][:],
                                    AF.Identity,
                                    bias=bk_s[:, j : j + 1],
                                )

                        # V (token-major with per-head ones column)
                        wv_t = ws.tile([P, HC, H], bf16, tag="wv", bufs=1)
                        nc.gpsimd.dma_start(
                            wv_t[:], t["Wv"].rearrange("(c p) m -> p c m", p=P)
                        )
                        for tt in range(TS):
                            ps = pp.tile([P, SQ], f32, tag="qkps")
                            for kc in range(HC):
                                mm(
                                    ps[:, 0:512],
                                    xT_s[:, kc, tt * P : (tt + 1) * P],
                                    wv_t[:, kc, 0:512],
                                    kc == 0,
                                    kc == HC - 1,
                                )
                                mm(
                                    ps[:, 512:H],
                                    xT_s[:, kc, tt * P : (tt + 1) * P],
                                    wv_t[:, kc, 512:H],
                                    kc == 0,
                                    kc == HC - 1,
                                )
                            ps_v = ps[:, 0:H].rearrange(
                                "p (j two d) -> p j two d", j=HC, two=2
                            )
                            bv_v = bvb_s[:].rearrange(
                                "p (j two d) -> p j two d", j=HC, two=2
                            )
                            nc.vector.scalar_tensor_tensor(
                                out=v_view[:, tt, :, 0:DH],
                                in0=ps_v[:, :, 0, :],
                                scalar=1.0,
                                in1=bv_v[:, :, 0, :],
                                op0=OP.mult,
                                op1=OP.add,
                            )
                            nc.vector.scalar_tensor_tensor(
                                out=v_view[:, tt, :, 97:161],
                                in0=ps_v[:, :, 1, :],
                                scalar=1.0,
                                in1=bv_v[:, :, 1, :],
                                op0=OP.mult,
                                op1=OP.add,
                            )

                    # ---------------- attention ----------------
                    with tc.tile_pool(name="attn_sb", bufs=1) as ab, tc.tile_pool(
                        name="probs", bufs=4
                    ) as prp, tc.tile_pool(
                        name="sc_ps", bufs=2, space="PSUM"
                    ) as pps, tc.tile_pool(
                        name="av_ps", bufs=2, space="PSUM"
                    ) as ppa:
                        avs = {}
                        spills = {}
                        bcs = {}

                        def spill_head(h):
                            # Copy the raw accumulator (attn rows + sums row,
                            # already partition-aligned with attnT) to SBUF on
                            # the DVE, compute the reciprocal row, broadcast
                            # it on GpSimd. For the last two heads the
                            # reciprocal runs on the (by-then idle) Act engine
                            # as exp(−ln d) — ~2.4µs instead of the 7.8µs DVE
                            # reciprocal, shortening the attention tail.
                            av = avs.pop(h)
                            avs_sb = ab.tile([P, SQ], f32, tag="avsb", bufs=3)
                            if h % 2 == 0:
                                dlo, dhi = 0, DH
                                drow = DH
                            else:
                                dlo, dhi = DH, P
                                drow = 32
                            nc.vector.tensor_copy(
                                avs_sb[dlo:dhi, :], av[dlo:dhi, :]
                            )
                            spills[h] = avs_sb
                            # the denominator row is read straight from PSUM
                            rec = ab.tile([1, SQ], f32r, tag="rec", bufs=3)
                            with nc.allow_low_precision(
                                reason="softmax recip rounding is benign"
                            ):
                                if h >= NH - 1:
                                    lnd = ab.tile(
                                        [1, SQ], f32, tag="lnd", bufs=2
                                    )
                                    nc.scalar.activation(
                                        lnd[:], av[drow : drow + 1, :],
                                        AF.Ln,
                                    )
                                    nc.scalar.activation(
                                        rec[:], lnd[:], AF.Exp, scale=-1.0
                                    )
                                else:
                                    nc.vector.reciprocal(
                                        rec[:], av[drow : drow + 1, :]
                                    )
                            bc = ab.tile([P, SQ], f32r, tag="bcs", bufs=3)
                            nc.gpsimd.partition_broadcast(
                                bc[:], rec[:], channels=P
                            )
                            bcs[h] = bc

                        def normalize_head(h):
                            """Divide head h's attention rows by the softmax
                            sums and place them into attnT. Emitted two heads
                            behind the matmul stream, entirely off the PE,
                            partition-aligned for both parities."""
                            hc = h // 2
                            avs_sb = spills.pop(h)
                            bc = bcs.pop(h)
                            if h % 2 == 0:
                                nc.vector.tensor_tensor(
                                    attnT_s[0:DH, hc, :], avs_sb[0:DH, :],
                                    bc[0:DH, :], OP.mult,
                                )
                            else:
                                nc.vector.tensor_tensor(
                                    attnT_s[DH:P, hc, :], avs_sb[DH:P, :],
                                    bc[DH:P, :], OP.mult,
                                )

                        def emit_av(h, av, kt, pr):
                            base = VB * (h // 2) + (0 if h % 2 == 0 else 33)
                            for n in range(2):
                                mm(
                                    av[:, n * 512 : (n + 1) * 512],
                                    v_s[:, kt, base : base + P],
                                    pr[:, n * 512 : (n + 1) * 512],
                                    kt == 0,
                                    kt == TS - 1,
                                )

                        for h in range(NH):
                            hc = h // 2
                            av = ppa.tile([P, SQ], f32, tag="av")
                            avs[h] = av
                            pending = []
                            for kt in range(TS):
                                sc = pps.tile([P, SQ], f32, tag="sc")
                                lhsT_k = kT_s[
                                    :, hc, kt * P : (kt + 1) * P
                                ]
                                for n in range(2):
                                    mm(
                                        sc[:, n * 512 : (n + 1) * 512],
                                        lhsT_k,
                                        qTz_s[
                                            :, h, n * 512 : (n + 1) * 512
                                        ],
                                        True,
                                        True,
                                    )
                                pr = prp.tile([P, SQ], bf16, tag="pr")
                                nc.scalar.activation(
                                    pr[:], sc[:], AF.Exp, bias=zero_s[:],
                                    scale=0.125,
                                )
                                pending.append((kt, pr))
                                if len(pending) > 2:
                                    emit_av(h, av, *pending.pop(0))
                            for p_ in pending:
                                emit_av(h, av, *p_)
                            spill_head(h)
                            if h >= 2:
                                normalize_head(h - 2)
                        normalize_head(NH - 2)
                        normalize_head(NH - 1)

                # ------------- O-projection + residual + LN1 -------------
                with tc.tile_pool(name="oproj", bufs=1) as op_, tc.tile_pool(
                    name="o_ps", bufs=2, space="PSUM"
                ) as ppo, tc.tile_pool(
                    name="st_ps", bufs=1, space="PSUM"
                ) as ppst:
                    nc.vector.tensor_scalar_mul(
                        l1wH_s[:], l1w_s[:], float(H)
                    )
                    nc.vector.tensor_scalar_mul(
                        l2wH_s[:], l2w_s[:], float(H)
                    )
                    r1_s = op_.tile([P, HC, SQ], bf16)
                    sum_ps = ppst.tile([1, SQ], f32, tag="lnsum", bufs=1)
                    sq_ps = ppst.tile([1, SQ], f32, tag="lnsq", bufs=1)
                    for j in range(HC):
                        wo_t = wop.tile([P, HC, P], bf16, tag="wo")
                        nc.gpsimd.dma_start(
                            wo_t[:],
                            t["Wo"][:, j * P : (j + 1) * P].rearrange(
                                "(c p) m -> p c m", p=P
                            ),
                        )
                        ps = ppo.tile([P, SQ], f32, tag="ops")
                        for kc in range(HC):
                            for n in range(2):
                                mm(
                                    ps[:, n * 512 : (n + 1) * 512],
                                    wo_t[:, kc, :],
                                    attnT_s[
                                        :, kc, n * 512 : (n + 1) * 512
                                    ],
                                    kc == 0,
                                    kc == HC - 1,
                                )
                        nc.vector.scalar_tensor_tensor(
                            out=r1_s[:, j, :],
                            in0=ps[:],
                            scalar=bo_s[:, j : j + 1],
                            in1=xT_s[:, j, 0:SQ],
                            op0=OP.add,
                            op1=OP.add,
                        )
                        # LN1 stats accumulate as the chunks appear
                        sq_t = op_.tile([P, SQ], bf16, tag="lnsqt", bufs=2)
                        nc.vector.tensor_tensor(
                            sq_t[:], r1_s[:, j, :], r1_s[:, j, :], OP.mult
                        )
                        for n in range(2):
                            mm(
                                sum_ps[:, n * 512 : (n + 1) * 512],
                                ones_s[:],
                                r1_s[:, j, n * 512 : (n + 1) * 512],
                                j == 0,
                                j == HC - 1,
                            )
                            mm(
                                sq_ps[:, n * 512 : (n + 1) * 512],
                                ones_s[:],
                                sq_t[:, n * 512 : (n + 1) * 512],
                                j == 0,
                                j == HC - 1,
                            )
                    def ln1_chunk(j, t2):
                        nc.scalar.activation(
                            x1b_s[:, j, :], t2[:], AF.Identity,
                            scale=l1wH_s[:, j : j + 1],
                            bias=l1b_s[:, j : j + 1],
                        )

                    # prefetch the first W1 chunks now: the GpSimd queue is
                    # about to fill with LN1 work, and MLP1's first matmuls
                    # should only wait on x1b chunks, not weights
                    w1_pre = []
                    for m in range(3):
                        w1_t = w1p.tile([P, HC, P], bf16, tag="w1")
                        nc.gpsimd.dma_start(
                            w1_t[:],
                            t["W1"][:, m * P : (m + 1) * P].rearrange(
                                "(c p) n -> p c n", p=P
                            ),
                        )
                        w1_pre.append(w1_t)

                    mb1, rb1 = ln_rows(op_, sum_ps, sq_ps)
                    ln_chunks(op_, r1_s, mb1, rb1, ln1_chunk)

        # ---------------- MLP + LN2 + output ----------------
        with tc.tile_pool(name="mlp", bufs=1) as mp:
            hT_s = mp.tile([P, FC, SQ], bf16)
            r2_s = mp.tile([P, HC, SQ], bf16)
            w2_s = mp.tile([P, FC, H], bf16)
            # W2 prefetch: issue early on the idle Sync queue, chunked so
            # the first MLP2 matmul doesn't wait on the whole 4.7MB.
            w2_src = t["W2"].rearrange("(c p) m -> p c m", p=P)
            for ci in range(4):
                nc.sync.dma_start(
                    w2_s[:, ci * 6 : (ci + 1) * 6, :],
                    w2_src[:, ci * 6 : (ci + 1) * 6, :],
                )
            with tc.tile_pool(
                name="m_ps", bufs=2, space="PSUM"
            ) as ppm, tc.tile_pool(
                name="st2_ps", bufs=1, space="PSUM"
            ) as ppst2:
                for m in range(FC):
                    if m < len(w1_pre):
                        w1_t = w1_pre[m]
                    else:
                        w1_t = w1p.tile([P, HC, P], bf16, tag="w1")
                        nc.gpsimd.dma_start(
                            w1_t[:],
                            t["W1"][:, m * P : (m + 1) * P].rearrange(
                                "(c p) n -> p c n", p=P
                            ),
                        )
                    ps = ppm.tile([P, SQ], f32, tag="mps")
                    for kc in range(HC):
                        for n in range(2):
                            mm(
                                ps[:, n * 512 : (n + 1) * 512],
                                w1_t[:, kc, :],
                                x1b_s[:, kc, n * 512 : (n + 1) * 512],
                                kc == 0,
                                kc == HC - 1,
                            )
                    nc.scalar.activation(
                        hT_s[:, m, :], ps[:], AF.Gelu, bias=b1_s[:, m : m + 1]
                    )

                # identity for the output transposes (gpsimd is idle here)
                make_identity(nc, ident_s[:])

                sum2_ps = ppst2.tile([1, SQ], f32, tag="ln2sum", bufs=1)
                sq2_ps = ppst2.tile([1, SQ], f32, tag="ln2sq", bufs=1)
                for j in range(HC):
                    ps = ppm.tile([P, SQ], f32, tag="mps")
                    for kc in range(FC):
                        for n in range(2):
                            mm(
                                ps[:, n * 512 : (n + 1) * 512],
                                w2_s[:, kc, j * P : (j + 1) * P],
                                hT_s[:, kc, n * 512 : (n + 1) * 512],
                                kc == 0,
                                kc == FC - 1,
                            )
                    nc.vector.scalar_tensor_tensor(
                        out=r2_s[:, j, :],
                        in0=ps[:],
                        scalar=b2_s[:, j : j + 1],
                        in1=x1b_s[:, j, :],
                        op0=OP.add,
                        op1=OP.add,
                    )
                    sq_t = mp.tile([P, SQ], bf16, tag="ln2sqt", bufs=1)
                    nc.vector.tensor_tensor(
                        sq_t[:], r2_s[:, j, :], r2_s[:, j, :], OP.mult
                    )
                    for n in range(2):
                        mm(
                            sum2_ps[:, n * 512 : (n + 1) * 512],
                            ones_s[:],
                            r2_s[:, j, n * 512 : (n + 1) * 512],
                            j == 0,
                            j == HC - 1,
                        )
                        mm(
                            sq2_ps[:, n * 512 : (n + 1) * 512],
                            ones_s[:],
                            sq_t[:, n * 512 : (n + 1) * 512],
                            j == 0,
                            j == HC - 1,
                        )
                mb2, rb2 = ln_rows(mp, sum2_ps, sq2_ps)

            # LN2 chunks + transpose back to token-major + store, pipelined
            # per feature chunk (the MLP psum pools are closed here, freeing
            # banks for the transpose pool)
            with tc.tile_pool(name="outp", bufs=1) as outp, tc.tile_pool(
                name="tr_ps", bufs=4, space="PSUM"
            ) as ppt:

                def ln2_chunk(j, t2):
                    r2n = outp.tile([P, SQ], bf16, tag="r2n", bufs=2)
                    nc.vector.tensor_scalar(
                        r2n[:], t2[:], l2wH_s[:, j : j + 1],
                        l2b_s[:, j : j + 1], OP.mult, OP.add,
                    )
                    stage = outp.tile([P, TQ, P], f32, tag="out", bufs=2)
                    for tp in range(TQ // 2):
                        tps = ppt.tile([P, 2, P], bf16, tag="tr")
                        for k in range(2):
                            nc.tensor.transpose(
                                tps[:, k, :],
                                r2n[:, (2 * tp + k) * P : (2 * tp + k + 1) * P],
                                ident_s[:],
                            )
                        nc.scalar.activation(
                            stage[:, 2 * tp : 2 * tp + 2, :], tps[:],
                            AF.Identity, bias=zero_s[:],
                        )
                    eng = nc.sync if j % 2 == 0 else nc.scalar
                    eng.dma_start(
                        t["y"][:, j * P : (j + 1) * P].rearrange(
                            "(t p) f -> p t f", p=P
                        ),
                        stage[:],
                    )

                ln_chunks(mp, r2_s, mb2, rb2, ln2_chunk)


def _build():
    import concourse.bacc as bacc
    import concourse.tile as tile
    import concourse.mybir as mybir
    from concourse.masks import make_identity

    f32 = mybir.dt.float32
    f32r = mybir.dt.float32r
    bf16 = mybir.dt.bfloat16

    nc = bacc.Bacc(
        "TRN2", target_bir_lowering=False, debug=False, num_devices=N_CORES
    )
    specs = [
        ("xT", [H, S], bf16, "ExternalInput"),
        ("Wq", [H, H], bf16, "ExternalInput"),
        ("Wk", [H, H], bf16, "ExternalInput"),
        ("Wv", [H, H], bf16, "ExternalInput"),
        ("Wo", [H, H], bf16, "ExternalInput"),
        ("W1", [H, FF], bf16, "ExternalInput"),
        ("W2", [FF, H], bf16, "ExternalInput"),
        ("bq2", [P, HC], f32, "ExternalInput"),
        ("bk2", [P, HC], f32, "ExternalInput"),
        ("bv", [H], f32, "ExternalInput"),
        ("bo2", [P, HC], f32, "ExternalInput"),
        ("b12", [P, FC], f32, "ExternalInput"),
        ("b22", [P, HC], f32, "ExternalInput"),
        ("l1w", [P, HC], f32, "ExternalInput"),
        ("l1b", [P, HC], f32, "ExternalInput"),
        ("l2w", [P, HC], f32, "ExternalInput"),
        ("l2b", [P, HC], f32, "ExternalInput"),
        ("y", [SQ, H], f32, "ExternalOutput"),
    ]
    t = {
        name: nc.dram_tensor(name, shape, dt, kind=kind).ap()
        for name, shape, dt, kind in specs
    }
    with tile.TileContext(nc) as tc:
        _emit(nc, tc, t, mybir, make_identity)
    nc.compile()
    return nc


def _chunk_major(v):
    """[C*P] -> [P, C] with entry [p, c] = v[c*P + p]."""
    return np.ascontiguousarray(v.reshape(-1, P).T)


def prepare_in_maps(inputs):
    inp = {k: np.asarray(v) for k, v in inputs.items()}
    x = inp["x"].astype(np.float32)

    shared = {
        "Wq": inp["Wq"].astype(BF16),
        "Wk": inp["Wk"].astype(BF16),
        "Wv": inp["Wv"].astype(BF16),
        "Wo": inp["Wo"].astype(BF16),
        "W1": inp["W1"].astype(BF16),
        "W2": inp["W2"].astype(BF16),
        "bq2": _chunk_major(inp["bq"].astype(np.float32)),
        "bk2": _chunk_major(inp["bk"].astype(np.float32)),
        "bv": inp["bv"].astype(np.float32),
        "bo2": _chunk_major(inp["bo"].astype(np.float32)),
        "b12": _chunk_major(inp["b1"].astype(np.float32)),
        "b22": _chunk_major(inp["b2"].astype(np.float32)),
        "l1w": _chunk_major(inp["ln1_w"].astype(np.float32)),
        "l1b": _chunk_major(inp["ln1_b"].astype(np.float32)),
        "l2w": _chunk_major(inp["ln2_w"].astype(np.float32)),
        "l2b": _chunk_major(inp["ln2_b"].astype(np.float32)),
    }
    in_maps = []
    for c in range(N_CORES):
        b, hf = c // 2, c % 2
        xT = x[b].T
        if hf:
            # rotate so this core's query half sits at columns 0:SQ (the
            # program is SPMD-identical; key order is irrelevant since
            # attention reduces over all keys)
            xT = np.concatenate([xT[:, SQ:], xT[:, :SQ]], axis=1)
        m = dict(shared)
        m["xT"] = np.ascontiguousarray(xT).astype(BF16)
        in_maps.append(m)
    return in_maps


def get_program():
    if "nc" not in _CACHE:
        _CACHE["nc"] = _build()
    return _CACHE["nc"]


def kernel(**inputs):
    from concourse.bass_utils import run_bass_kernel_spmd

    nc = get_program()
    in_maps = prepare_in_maps(inputs)
    res = run_bass_kernel_spmd(nc, in_maps, core_ids=list(range(N_CORES)))
    out = np.empty((B, S, H), np.float32)
    for c in range(N_CORES):
        b, hf = c // 2, c % 2
        out[b, hf * SQ : (hf + 1) * SQ] = res.results[c]["y"]
    return out
